# revision 13
# baseline (speedup 1.0000x reference)
"""GCN encoder (6-layer) on 8 Trainium2 NeuronCores.

Strategy: the sparse GCN aggregation  agg = segment_sum(norm * m[src], dst)
is a fixed sparse-matrix product  agg = A @ m  with
A = D^-1/2 (Adj + I) D^-1/2  (10000x10000, ~0.33% dense, unstructured).
On the 128x128 PE array the dense formulation wins: nodes are sharded
1250 (padded to 1280) per core; each core streams its [10240 x 1280] A^T
shard (bf16, 26 MB) from HBM each layer while accumulating
agg_part = A_part @ m_full in PSUM (fp32).  m_full is rebuilt each layer
via a bf16 AllGather of the per-core GEMM1 results.  Epilogue
(bias + exact-erf GELU + LayerNorm + residual) runs on ACT/DVE, fully
overlapped with the PE stream.  LayerNorm rsqrt is computed on DVE
(Newton iterations over an exponent-hack seed) so the ACT table set
never leaves `gelu_and_others`.
"""

import math
import numpy as np
import ml_dtypes

import bass_rust
import concourse.bass as bass
import concourse.mybir as mybir
import concourse.tile as tile
from concourse.vector_clock import ScopedClock
from concourse.masks import make_identity

F32 = mybir.dt.float32
BF16 = mybir.dt.bfloat16
AF = mybir.ActivationFunctionType
ALU = mybir.AluOpType

# ---------------------------------------------------------------- config

class Cfg:
    def __init__(self, n_real=10000, mt=10, kg=20, l=6, h=256, in_dim=128,
                 alpha=0.1, eps=1e-5, ncores=8):
        self.P = 128
        self.NCORES = ncores
        self.MT = mt                      # m-tiles (128 rows) per core
        self.NPC = mt * 128               # padded nodes per core
        self.NPAD = self.NPC * ncores     # padded total nodes
        self.KT = self.NPAD // 128        # k-tiles in the big matmul
        self.KG = kg                      # k-tiles per A^T DMA group
        assert self.KT % (ncores * mt // ncores) == 0
        self.G = self.KT // kg            # DMA groups per chunk
        assert self.KT % kg == 0
        assert mt % 2 == 0
        self.MC = mt // 2                 # chunks of 2 m-tiles
        self.L = l
        self.H = h
        self.HT = h // 128                # h-tiles (2)
        self.IN = in_dim
        self.INT = in_dim // 128          # input k-tiles (1)
        self.N = n_real
        self.RPC = (n_real + ncores - 1) // ncores  # real rows per core
        assert self.RPC <= self.NPC
        self.ALPHA = alpha
        self.EPS = eps
        self.ACT = AF.Gelu  # sim test overrides (Gelu not implemented in sim)


# ------------------------------------------------- drain-wait workaround

class SplitDrainTileContext(tile.TileContext):
    """This walrus build rejects >1 sync-wait on a CTRL (Drain) instruction;
    Tile's kernel-tail drain accumulates one wait per logical processor.
    Split the waits across a chain of drain instructions."""

    DRAIN_WAIT_CAP = 1

    def _drain_and_barrier(self, tick_clock, wait_clock):
        drain_inst = self.nc.sync.drain()
        wait_clock.add_sem_waits(
            drain_inst.ins, ScopedClock({None: tick_clock.global_clock})
        )
        si = drain_inst.ins.sync_info
        if si is not None:
            waits = list(si.on_wait)
            ups = list(si.on_update)
            cap = self.DRAIN_WAIT_CAP
            if len(waits) > cap:
                drain_inst.ins.sync_info = bass_rust.SyncInfo(
                    on_wait=waits[:cap], on_update=ups
                )
                rest = waits[cap:]
                for i in range(0, len(rest), cap):
                    d = self.nc.sync.drain()
                    d.ins.sync_info = bass_rust.SyncInfo(
                        on_wait=rest[i:i + cap], on_update=[]
                    )
        self.nc.all_engine_barrier()
        assert self.sems is not None
        popped = self.nc._tile_sem_poison_stack.pop()
        assert popped is self._sem_poison
        self.nc.clear_and_free_semaphores(list(self.sems.allocated().values()))
        self.nc.all_engine_barrier()


# This walrus build caps sync-waits at 1 per instruction. Tile packs one wait
# per producer proc onto consumer instructions. Rewrite:
#  - engine-executed instructions: move excess waits onto same-engine NoOps
#    inserted just before the instruction (engine subsequence order preserved)
#  - DMACopy (queue-executed -- a NoOp cannot sit in a DGE queue): move ALL its
#    waits onto an SP NoOp chain whose last link bumps a helper semaphore; the
#    DMA then waits only `helper >= k`. Safe because every producer of the
#    moved waits is scheduled before this program point, so blocking SP here
#    cannot deadlock (SP has already pushed all earlier descriptors).
_SEM_CHAIN_OPCODES = {"DMACopy", "TriggerCollective", "CollectiveCompute"}


def split_excess_waits(nc, helper, cap=1):
    fn = nc.m.functions[0]
    ctr = 0
    kval = 0
    sp = mybir.EngineType.SP
    used_helper = False
    for bb in fn.blocks:
        out = []
        changed = False
        for inst in bb.instructions:
            si = inst.sync_info
            n_w = len(si.on_wait) if si is not None else 0
            if n_w > cap and inst.opcode not in _SEM_CHAIN_OPCODES:
                waits = list(si.on_wait)
                extra = waits[cap:]
                for j in range(0, len(extra), cap):
                    ctr += 1
                    n = bass_rust.InstNoOp(name=f"wsplit-{ctr}", ins=[], outs=[])
                    n.engine = inst.engine
                    n.bass_nofuse = True
                    n.sync_info = bass_rust.SyncInfo(
                        on_wait=extra[j:j + cap], on_update=[])
                    out.append(n)
                inst.sync_info = bass_rust.SyncInfo(
                    on_wait=waits[:cap], on_update=list(si.on_update))
                changed = True
            elif n_w > cap:
                # queue-executed: SP NoOp chain, one wait each; last bumps the
                # helper; the instruction waits helper >= kval only.
                waits = list(si.on_wait)
                kval += 1
                used_helper = True
                for j, w in enumerate(waits):
                    ctr += 1
                    n = bass_rust.InstNoOp(name=f"wsplit-{ctr}", ins=[], outs=[])
                    n.engine = sp
                    n.bass_nofuse = True
                    ups = []
                    if j == len(waits) - 1:
                        ups = [bass_rust.SyncUpdate(
                            ant_name=helper.name, id=helper.num,
                            sync_type="semaphore", update_mode="sem-inc",
                            update_value=1)]
                    n.sync_info = bass_rust.SyncInfo(on_wait=[w], on_update=ups)
                    out.append(n)
                hw = bass_rust.SyncWait(
                    ant_name=helper.name, id=helper.num, sync_type="semaphore",
                    wait_mode="sem-ge-imm", wait_value=kval)
                inst.sync_info = bass_rust.SyncInfo(
                    on_wait=[hw], on_update=list(si.on_update))
                changed = True
            out.append(inst)
        if changed:
            bb.instructions = out
    if used_helper:
        # reset for any later execution of the NEFF (NRT does not zero kernel
        # semaphores between executions; Tile clears only its own)
        nc.sync.sem_clear(helper)
    return ctr


# ---------------------------------------------------------- device kernel

def build_nc(cfg: Cfg, split_waits=True):
    c = cfg
    nc = bass.Bass("TRN2", target_bir_lowering=False, debug=False,
                   num_devices=c.NCORES)
    # reserved before TileContext so Tile can never hand out the same sem id
    wsplit_sem = nc.alloc_semaphore("wsplit_dma") if split_waits else None

    # ---- I/O ----
    xT_d = nc.dram_tensor("xT", [c.P, c.INT * c.NPC], F32, kind="ExternalInput").ap()
    At_d = nc.dram_tensor("At", [c.MC, c.G, c.P, c.KG * 256], BF16,
                          kind="ExternalInput").ap()
    win_d = nc.dram_tensor("Win", [c.P, c.INT * c.H], F32, kind="ExternalInput").ap()
    wl_d = nc.dram_tensor("Wlh", [c.P, c.L * c.HT * c.H], F32,
                          kind="ExternalInput").ap()
    cin_d = nc.dram_tensor("cin", [c.P, 3 * c.H], F32, kind="ExternalInput").ap()
    cl_d = nc.dram_tensor("cl", [c.P, 3 * c.L * c.H], F32, kind="ExternalInput").ap()
    out_d = nc.dram_tensor("out", [c.NPC, c.H], F32, kind="ExternalOutput").ap()

    # collective bounce buffers (per layer)
    cc_in = [nc.dram_tensor(f"cc_in_{l}", [c.P, c.MT * c.H], BF16)
             for l in range(c.L)]
    cc_out = [nc.dram_tensor(f"cc_out_{l}", [c.P * c.NCORES, c.MT * c.H], BF16,
                             addr_space="Shared")
              for l in range(c.L)]
    rg = [list(range(c.NCORES))]

    with SplitDrainTileContext(nc) as tc:
        with (
            tc.tile_pool(name="const", bufs=1) as const,
            tc.tile_pool(name="state", bufs=1) as state,
            tc.tile_pool(name="at", bufs=3) as atp,
            tc.tile_pool(name="tmp", bufs=4) as tmp,
            tc.tile_pool(name="stat", bufs=4) as statp,
            tc.tile_pool(name="agg", bufs=4, space="PSUM") as aggp,
            tc.tile_pool(name="g1", bufs=2, space="PSUM") as g1p,
            tc.tile_pool(name="tp", bufs=2, space="PSUM") as tpp,
        ):
            # ---- constants ----
            ident = const.tile([c.P, c.P], F32)
            make_identity(nc, ident)
            xT = const.tile([c.P, c.INT * c.NPC], F32)
            nc.sync.dma_start(out=xT, in_=xT_d)
            win = const.tile([c.P, c.INT * c.H], F32)
            nc.sync.dma_start(out=win, in_=win_d)
            wl = const.tile([c.P, c.L * c.HT * c.H], F32)
            nc.sync.dma_start(out=wl, in_=wl_d)
            cin = const.tile([c.P, 3 * c.H], F32)     # b_in | g_in | beta_in bcast
            nc.sync.dma_start(out=cin, in_=cin_d)
            cl = const.tile([c.P, 3 * c.L * c.H], F32)  # bl | 0.9gl | 0.9betal bcast
            nc.sync.dma_start(out=cl, in_=cl_d)

            # ---- persistent state ----
            cur = state.tile([c.P, c.MT * c.H], F32)
            h0 = state.tile([c.P, c.MT * c.H], F32)
            curT = state.tile([c.P, c.HT * c.NPC], F32)
            mpart = state.tile([c.P, c.MT * c.H], BF16)
            mfull = state.tile([c.P, c.KT * c.H], BF16)

            H = c.H

            def rsqrt_dve(out, ve):
                """out = (ve)^-0.5 on DVE only: exponent-hack seed + 3 Newton
                iterations. ve is [128, n] f32, strictly positive."""
                n = ve.shape[-1]
                i32 = statp.tile([c.P, n], mybir.dt.int32, tag="rs_i")
                # i = ve_bits >> 1
                nc.vector.tensor_scalar(out=i32, in0=ve.bitcast(mybir.dt.int32),
                                        scalar1=1, scalar2=None,
                                        op0=ALU.logical_shift_right)
                # i = 0x5f3759df - i  ==  i * (-1) + 0x5f3759df
                nc.vector.tensor_scalar(out=i32, in0=i32, scalar1=-1,
                                        scalar2=0x5F3759DF, op0=ALU.mult,
                                        op1=ALU.add)
                y = statp.tile([c.P, n], F32, tag="rs_y")
                nc.vector.tensor_copy(out=y, in_=i32.bitcast(F32))
                w = statp.tile([c.P, n], F32, tag="rs_w")
                for _ in range(3):
                    # w = ve * y * y ; y = y * (1.5 - 0.5 w)
                    nc.vector.tensor_tensor(out=w, in0=y, in1=y, op=ALU.mult)
                    nc.vector.tensor_tensor(out=w, in0=w, in1=ve, op=ALU.mult)
                    nc.vector.tensor_scalar(out=w, in0=w, scalar1=-0.5,
                                            scalar2=1.5, op0=ALU.mult, op1=ALU.add)
                    nc.vector.tensor_tensor(out=y, in0=y, in1=w, op=ALU.mult)
                nc.vector.tensor_copy(out=out, in_=y)

            def epilogue(m, ps, bias, gain, beta, first):
                """LN(gelu(ps + bias)) * gain + beta; first block writes h0/cur,
                layers do the residual update. Also refreshes curT for tile m."""
                t1 = tmp.tile([c.P, H], F32, tag="t1")
                nc.vector.tensor_tensor(out=t1, in0=ps, in1=bias, op=ALU.add)
                t2 = tmp.tile([c.P, H], F32, tag="t2")
                nc.scalar.activation(out=t2, in_=t1, func=c.ACT)
                st6 = statp.tile([c.P, 6], F32, tag="st6")
                nc.vector.bn_stats(out=st6, in_=t2)
                mv = statp.tile([c.P, 2], F32, tag="mv")
                nc.vector.bn_aggr(out=mv, in_=st6)
                ve = statp.tile([c.P, 1], F32, tag="ve")
                nc.vector.tensor_scalar_add(ve, mv[:, 1:2], c.EPS)
                rinv = statp.tile([c.P, 1], F32, tag="rinv")
                rsqrt_dve(rinv, ve)
                # z = (t2 - mean) * rinv
                z = tmp.tile([c.P, H], F32, tag="z")
                nc.vector.tensor_scalar(out=z, in0=t2, scalar1=mv[:, 0:1],
                                        scalar2=rinv, op0=ALU.subtract,
                                        op1=ALU.mult)
                nc.vector.tensor_tensor(out=z, in0=z, in1=gain, op=ALU.mult)
                cs = cur[:, m * H:(m + 1) * H]
                if first:
                    h0s = h0[:, m * H:(m + 1) * H]
                    nc.vector.tensor_tensor(out=h0s, in0=z, in1=beta, op=ALU.add)
                    nc.vector.tensor_copy(out=cs, in_=h0s)
                else:
                    nc.vector.tensor_tensor(out=z, in0=z, in1=beta, op=ALU.add)
                    # base = 0.1*h0 + cur ; cur = z + base
                    base = tmp.tile([c.P, H], F32, tag="base")
                    nc.vector.scalar_tensor_tensor(
                        out=base, in0=h0[:, m * H:(m + 1) * H], scalar=c.ALPHA,
                        in1=cs, op0=ALU.mult, op1=ALU.add)
                    nc.vector.tensor_tensor(out=cs, in0=z, in1=base, op=ALU.add)

            def refresh_curT(m):
                for t in range(c.HT):
                    pt = tpp.tile([c.P, c.P], F32, tag="tp")
                    nc.tensor.transpose(
                        pt, cur[:, m * H + t * 128: m * H + t * 128 + 128], ident)
                    nc.vector.tensor_copy(
                        out=curT[:, t * c.NPC + m * 128: t * c.NPC + (m + 1) * 128],
                        in_=pt)

            # ================= input block =================
            for m in range(c.MT):
                ps = g1p.tile([c.P, H], F32, tag="g1")
                for t in range(c.INT):
                    nc.tensor.matmul(
                        ps,
                        lhsT=xT[:, t * c.NPC + m * 128: t * c.NPC + (m + 1) * 128],
                        rhs=win[:, t * H:(t + 1) * H],
                        start=(t == 0), stop=(t == c.INT - 1))
                epilogue(m, ps, cin[:, 0:H], cin[:, H:2 * H], cin[:, 2 * H:3 * H],
                         first=True)
                refresh_curT(m)

            # ================= layers =================
            for l in range(c.L):
                # GEMM1 (fp32): mpart[m] = cur @ Wl[l], cast bf16
                for m in range(c.MT):
                    ps = g1p.tile([c.P, H], F32, tag="g1")
                    for t in range(c.HT):
                        nc.tensor.matmul(
                            ps,
                            lhsT=curT[:, t * c.NPC + m * 128:
                                      t * c.NPC + (m + 1) * 128],
                            rhs=wl[:, (l * c.HT + t) * H:(l * c.HT + t + 1) * H],
                            start=(t == 0), stop=(t == c.HT - 1))
                    nc.vector.tensor_copy(out=mpart[:, m * H:(m + 1) * H], in_=ps)
                nc.sync.dma_start(out=cc_in[l].ap(), in_=mpart)
                nc.gpsimd.collective_compute(
                    "AllGather", ALU.bypass, replica_groups=rg,
                    ins=[cc_in[l].ap()], outs=[cc_out[l].ap()])
                rk = c.MT * c.H  # columns contributed per rank
                for r in range(c.NCORES):
                    nc.sync.dma_start(
                        out=mfull[:, r * rk:(r + 1) * rk],
                        in_=cc_out[l].ap()[r * c.P:(r + 1) * c.P, :])

                # big matmul: agg_part = A_part @ m_full
                lb = cl[:, (3 * l) * H:(3 * l + 1) * H]
                lg = cl[:, (3 * l + 1) * H:(3 * l + 2) * H]
                lbeta = cl[:, (3 * l + 2) * H:(3 * l + 3) * H]
                for ch in range(c.MC):
                    atb = []
                    for g in range(c.G):
                        t = atp.tile([c.P, c.KG * 256], BF16, tag="at")
                        nc.sync.dma_start(out=t, in_=At_d[ch, g])
                        atb.append(t)
                    ps0 = aggp.tile([c.P, H], F32, tag="agg")
                    ps1 = aggp.tile([c.P, H], F32, tag="agg")
                    for g in range(c.G):
                        for kk in range(c.KG):
                            k = g * c.KG + kk
                            rhs = mfull[:, k * H:(k + 1) * H]
                            nc.tensor.matmul(
                                ps0, lhsT=atb[g][:, kk * 256: kk * 256 + 128],
                                rhs=rhs, start=(k == 0), stop=(k == c.KT - 1))
                            nc.tensor.matmul(
                                ps1, lhsT=atb[g][:, kk * 256 + 128: kk * 256 + 256],
                                rhs=rhs, start=(k == 0), stop=(k == c.KT - 1))
                    for j, ps in ((0, ps0), (1, ps1)):
                        m = 2 * ch + j
                        epilogue(m, ps, lb, lg, lbeta, first=False)
                        if l < c.L - 1:
                            refresh_curT(m)

            # ================= output =================
            out_v = out_d.rearrange("(m p) q -> p m q", p=c.P)
            nc.sync.dma_start(out=out_v, in_=cur.rearrange("p (m q) -> p m q", q=H))

    if split_waits:
        split_excess_waits(nc, wsplit_sem)
    return nc


# ---------------------------------------------------------- host wrapper

def prep_inputs(cfg, x, edge_index, W_in, b_in, g_in, beta_in, Wl, bl, gl, betal):
    """Build the per-core input maps (numpy, host-side)."""
    c = cfg
    x = np.asarray(x, dtype=np.float32)
    edge_index = np.asarray(edge_index)
    W_in = np.asarray(W_in, dtype=np.float32)
    b_in = np.asarray(b_in, dtype=np.float32)
    g_in = np.asarray(g_in, dtype=np.float32)
    beta_in = np.asarray(beta_in, dtype=np.float32)
    Wl = np.asarray(Wl, dtype=np.float32)
    bl = np.asarray(bl, dtype=np.float32)
    gl = np.asarray(gl, dtype=np.float32)
    betal = np.asarray(betal, dtype=np.float32)

    N = c.N
    # GCN normalization with self-loops
    src = np.concatenate([edge_index[0], np.arange(N, dtype=np.int64)])
    dst = np.concatenate([edge_index[1], np.arange(N, dtype=np.int64)])
    deg = np.bincount(dst, minlength=N).astype(np.float32)
    dinv = np.where(deg > 0, deg ** -0.5, 0.0).astype(np.float32)
    w = dinv[dst] * dinv[src]

    # padded global ids: node n -> core r = n // RPC, slot i = n % RPC
    nn = np.arange(N, dtype=np.int64)
    gid = (nn // c.RPC) * c.NPC + (nn % c.RPC)
    A = np.zeros((c.NPAD, c.NPAD), dtype=np.float32)
    np.add.at(A, (gid[dst], gid[src]), w)
    A16 = A.astype(ml_dtypes.bfloat16)
    del A

    # per-layer constants, broadcast across partitions
    def bcast(v):  # [H] -> [128, H]
        return np.broadcast_to(np.asarray(v, np.float32), (c.P, v.shape[-1])).copy()

    cin = np.concatenate([bcast(b_in), bcast(g_in), bcast(beta_in)], axis=1)
    cl_list = []
    for l in range(c.L):
        cl_list += [bcast(bl[l]), bcast((1.0 - c.ALPHA) * gl[l]),
                    bcast((1.0 - c.ALPHA) * betal[l])]
    cl_h = np.concatenate(cl_list, axis=1)

    # Wl host layout: [128, L*HT*H], k-tile t of layer l at cols (l*HT+t)*H
    wl_h = np.zeros((c.P, c.L * c.HT * c.H), np.float32)
    for l in range(c.L):
        for t in range(c.HT):
            wl_h[:, (l * c.HT + t) * c.H:(l * c.HT + t + 1) * c.H] = \
                Wl[l][t * 128:(t + 1) * 128, :]

    win_h = np.zeros((c.P, c.INT * c.H), np.float32)
    for t in range(c.INT):
        win_h[:, t * c.H:(t + 1) * c.H] = W_in[t * 128:(t + 1) * 128, :]

    in_maps = []
    for r in range(c.NCORES):
        lo, hi = r * c.RPC, min((r + 1) * c.RPC, N)
        xp = np.zeros((c.NPC, c.IN), np.float32)
        xp[:hi - lo] = x[lo:hi]
        xT = np.zeros((c.P, c.INT * c.NPC), np.float32)
        for t in range(c.INT):
            xT[:, t * c.NPC:(t + 1) * c.NPC] = xp[:, t * 128:(t + 1) * 128].T

        blk = A16[r * c.NPC:(r + 1) * c.NPC, :]          # [NPC, NPAD] (dst, src)
        t = np.ascontiguousarray(blk.T)                   # [NPAD, NPC] (src, dst)
        # -> [MC, G, P, KG*256]: chunk ch covers dst cols ch*256..,
        # group g covers k-tiles g*KG..
        at = t.reshape(c.G, c.KG, 128, c.MC, 256)
        at = at.transpose(3, 0, 2, 1, 4).reshape(c.MC, c.G, 128, c.KG * 256)
        at = np.ascontiguousarray(at)

        in_maps.append({
            "xT": xT, "At": at, "Win": win_h, "Wlh": wl_h,
            "cin": cin, "cl": cl_h,
        })
    return in_maps


def postprocess(cfg, results):
    c = cfg
    out = np.empty((c.N, c.H), np.float32)
    for r in range(c.NCORES):
        lo, hi = r * c.RPC, min((r + 1) * c.RPC, c.N)
        out[lo:hi] = results[r]["out"][:hi - lo]
    return out


_CACHE = {}
TRACE = False  # test harness sets True to capture an NTFF profile


def kernel(x, edge_index, W_in, b_in, g_in, beta_in, Wl, bl, gl, betal):
    from concourse import bass_utils
    cfg = Cfg()
    in_maps = prep_inputs(cfg, x, edge_index, W_in, b_in, g_in, beta_in,
                          Wl, bl, gl, betal)
    if "nc" not in _CACHE:
        _CACHE["nc"] = build_nc(cfg)
    res = bass_utils.run_bass_kernel_spmd(
        _CACHE["nc"], in_maps, core_ids=list(range(cfg.NCORES)), trace=TRACE)
    _CACHE["last_result"] = res
    return postprocess(cfg, res.results)


# revision 18
# speedup vs baseline: 1.0291x; 1.0291x over previous
"""GCN encoder (6-layer) on 8 Trainium2 NeuronCores.

Strategy: the sparse GCN aggregation  agg = segment_sum(norm * m[src], dst)
is a fixed sparse-matrix product  agg = A @ m  with
A = D^-1/2 (Adj + I) D^-1/2  (10000x10000, ~0.33% dense, unstructured).
On the 128x128 PE array the dense formulation wins: nodes are sharded
1250 (padded to 1280) per core; each core streams its [10240 x 1280] A^T
shard (bf16, 26 MB) from HBM each layer while accumulating
agg_part = A_part @ m_full in PSUM (fp32).  m_full is rebuilt each layer
via a bf16 AllGather of the per-core GEMM1 results.  Epilogue
(bias + exact-erf GELU + LayerNorm + residual) runs on ACT/DVE, fully
overlapped with the PE stream.  LayerNorm rsqrt is computed on DVE
(Newton iterations over an exponent-hack seed) so the ACT table set
never leaves `gelu_and_others`.
"""

import math
import numpy as np
import ml_dtypes

import bass_rust
import concourse.bass as bass
import concourse.mybir as mybir
import concourse.tile as tile
from concourse.vector_clock import ScopedClock
from concourse.masks import make_identity

F32 = mybir.dt.float32
BF16 = mybir.dt.bfloat16
AF = mybir.ActivationFunctionType
ALU = mybir.AluOpType

# ---------------------------------------------------------------- config

class Cfg:
    def __init__(self, n_real=10000, mt=10, kg=20, l=6, h=256, in_dim=128,
                 alpha=0.1, eps=1e-5, ncores=8):
        self.P = 128
        self.NCORES = ncores
        self.MT = mt                      # m-tiles (128 rows) per core
        self.NPC = mt * 128               # padded nodes per core
        self.NPAD = self.NPC * ncores     # padded total nodes
        self.KT = self.NPAD // 128        # k-tiles in the big matmul
        self.KG = kg                      # k-tiles per A^T DMA group
        assert self.KT % (ncores * mt // ncores) == 0
        self.G = self.KT // kg            # DMA groups per chunk
        assert self.KT % kg == 0
        assert mt % 2 == 0
        self.MC = mt // 2                 # chunks of 2 m-tiles
        self.L = l
        self.H = h
        self.HT = h // 128                # h-tiles (2)
        self.IN = in_dim
        self.INT = in_dim // 128          # input k-tiles (1)
        self.N = n_real
        self.RPC = (n_real + ncores - 1) // ncores  # real rows per core
        assert self.RPC <= self.NPC
        self.ALPHA = alpha
        self.EPS = eps
        self.ACT = AF.Gelu  # sim test overrides (Gelu not implemented in sim)
        # AllGather split points (m-tile ranges); later parts are smaller so
        # the tail collective hides under the next layer's phase-a matmuls
        if mt >= 6:
            self.SPLITS = [(0, mt - 4), (mt - 4, mt - 2), (mt - 2, mt)]
        elif mt == 4:
            self.SPLITS = [(0, 2), (2, 3), (3, 4)]
        else:
            self.SPLITS = [(0, 1), (1, 2)]
        # global k-tile order of the big matmul: part-major, rank-major inside
        self.PERM = [r * mt + m
                     for (s, e) in self.SPLITS
                     for r in range(ncores)
                     for m in range(s, e)]
        # part index of each k-position
        self.KPART = []
        self.KOFF = []  # column offset (in k-tiles) inside that part's mfull
        off = 0
        for p, (s, e) in enumerate(self.SPLITS):
            n = ncores * (e - s)
            self.KPART += [p] * n
            self.KOFF += list(range(n))
            off += n


# ------------------------------------------------- drain-wait workaround

class SplitDrainTileContext(tile.TileContext):
    """This walrus build rejects >1 sync-wait on a CTRL (Drain) instruction;
    Tile's kernel-tail drain accumulates one wait per logical processor.
    Split the waits across a chain of drain instructions."""

    DRAIN_WAIT_CAP = 1

    def _drain_and_barrier(self, tick_clock, wait_clock):
        drain_inst = self.nc.sync.drain()
        wait_clock.add_sem_waits(
            drain_inst.ins, ScopedClock({None: tick_clock.global_clock})
        )
        si = drain_inst.ins.sync_info
        if si is not None:
            waits = list(si.on_wait)
            ups = list(si.on_update)
            cap = self.DRAIN_WAIT_CAP
            if len(waits) > cap:
                drain_inst.ins.sync_info = bass_rust.SyncInfo(
                    on_wait=waits[:cap], on_update=ups
                )
                rest = waits[cap:]
                for i in range(0, len(rest), cap):
                    d = self.nc.sync.drain()
                    d.ins.sync_info = bass_rust.SyncInfo(
                        on_wait=rest[i:i + cap], on_update=[]
                    )
        self.nc.all_engine_barrier()
        assert self.sems is not None
        popped = self.nc._tile_sem_poison_stack.pop()
        assert popped is self._sem_poison
        self.nc.clear_and_free_semaphores(list(self.sems.allocated().values()))
        self.nc.all_engine_barrier()


# This walrus build caps sync-waits at 1 per instruction. Tile packs one wait
# per producer proc onto consumer instructions. Rewrite:
#  - engine-executed instructions: move excess waits onto same-engine NoOps
#    inserted just before the instruction (engine subsequence order preserved)
#  - DMACopy (queue-executed -- a NoOp cannot sit in a DGE queue): move ALL its
#    waits onto an SP NoOp chain whose last link bumps a helper semaphore; the
#    DMA then waits only `helper >= k`. Safe because every producer of the
#    moved waits is scheduled before this program point, so blocking SP here
#    cannot deadlock (SP has already pushed all earlier descriptors).
_SEM_CHAIN_OPCODES = {"DMACopy", "TriggerCollective", "CollectiveCompute"}


def split_excess_waits(nc, helper, cap=1):
    fn = nc.m.functions[0]
    ctr = 0
    kval = 0
    sp = mybir.EngineType.SP
    used_helper = False
    for bb in fn.blocks:
        out = []
        changed = False
        for inst in bb.instructions:
            si = inst.sync_info
            n_w = len(si.on_wait) if si is not None else 0
            if n_w > cap and inst.opcode not in _SEM_CHAIN_OPCODES:
                waits = list(si.on_wait)
                extra = waits[cap:]
                for j in range(0, len(extra), cap):
                    ctr += 1
                    n = bass_rust.InstNoOp(name=f"wsplit-{ctr}", ins=[], outs=[])
                    n.engine = inst.engine
                    n.bass_nofuse = True
                    n.sync_info = bass_rust.SyncInfo(
                        on_wait=extra[j:j + cap], on_update=[])
                    out.append(n)
                inst.sync_info = bass_rust.SyncInfo(
                    on_wait=waits[:cap], on_update=list(si.on_update))
                changed = True
            elif n_w > cap:
                # queue-executed: SP NoOp chain, one wait each; last bumps the
                # helper; the instruction waits helper >= kval only.
                waits = list(si.on_wait)
                kval += 1
                used_helper = True
                for j, w in enumerate(waits):
                    ctr += 1
                    n = bass_rust.InstNoOp(name=f"wsplit-{ctr}", ins=[], outs=[])
                    n.engine = sp
                    n.bass_nofuse = True
                    ups = []
                    if j == len(waits) - 1:
                        ups = [bass_rust.SyncUpdate(
                            ant_name=helper.name, id=helper.num,
                            sync_type="semaphore", update_mode="sem-inc",
                            update_value=1)]
                    n.sync_info = bass_rust.SyncInfo(on_wait=[w], on_update=ups)
                    out.append(n)
                hw = bass_rust.SyncWait(
                    ant_name=helper.name, id=helper.num, sync_type="semaphore",
                    wait_mode="sem-ge-imm", wait_value=kval)
                inst.sync_info = bass_rust.SyncInfo(
                    on_wait=[hw], on_update=list(si.on_update))
                changed = True
            out.append(inst)
        if changed:
            bb.instructions = out
    if used_helper:
        # reset for any later execution of the NEFF (NRT does not zero kernel
        # semaphores between executions; Tile clears only its own)
        nc.sync.sem_clear(helper)
    return ctr


# ---------------------------------------------------------- device kernel

def build_nc(cfg: Cfg, split_waits=True):
    c = cfg
    nc = bass.Bass("TRN2", target_bir_lowering=False, debug=False,
                   num_devices=c.NCORES)
    # reserved before TileContext so Tile can never hand out the same sem id
    wsplit_sem = nc.alloc_semaphore("wsplit_dma") if split_waits else None

    # ---- I/O ----
    xT_d = nc.dram_tensor("xT", [c.P, c.INT * c.NPC], F32, kind="ExternalInput").ap()
    At_d = nc.dram_tensor("At", [c.MC, c.G, c.P, c.KG * 256], BF16,
                          kind="ExternalInput").ap()
    win_d = nc.dram_tensor("Win", [c.P, c.INT * c.H], F32, kind="ExternalInput").ap()
    wl_d = nc.dram_tensor("Wlh", [c.P, c.L * c.HT * c.H], F32,
                          kind="ExternalInput").ap()
    cin_d = nc.dram_tensor("cin", [c.P, 3 * c.H], F32, kind="ExternalInput").ap()
    cl_d = nc.dram_tensor("cl", [c.P, 3 * c.L * c.H], F32, kind="ExternalInput").ap()
    out_d = nc.dram_tensor("out", [c.NPC, c.H], F32, kind="ExternalOutput").ap()

    # collective bounce buffers (per layer, per AllGather part)
    nparts = len(c.SPLITS)
    cc_in = [[nc.dram_tensor(f"cc_in_{l}_{p}", [c.P, (e - s) * c.H], BF16)
              for p, (s, e) in enumerate(c.SPLITS)] for l in range(c.L)]
    cc_out = [[nc.dram_tensor(f"cc_out_{l}_{p}",
                              [c.P * c.NCORES, (e - s) * c.H], BF16,
                              addr_space="Shared")
               for p, (s, e) in enumerate(c.SPLITS)] for l in range(c.L)]
    rg = [list(range(c.NCORES))]

    with SplitDrainTileContext(nc) as tc:
        with (
            tc.tile_pool(name="const", bufs=1) as const,
            tc.tile_pool(name="state", bufs=1) as state,
            tc.tile_pool(name="at", bufs=3) as atp,
            tc.tile_pool(name="tmp", bufs=4) as tmp,
            tc.tile_pool(name="stat", bufs=4) as statp,
            tc.tile_pool(name="agg", bufs=4, space="PSUM") as aggp,
            tc.tile_pool(name="g1", bufs=2, space="PSUM") as g1p,
            tc.tile_pool(name="tp", bufs=2, space="PSUM") as tpp,
        ):
            # ---- constants ----
            ident = const.tile([c.P, c.P], F32)
            make_identity(nc, ident)
            xT = const.tile([c.P, c.INT * c.NPC], F32)
            nc.sync.dma_start(out=xT, in_=xT_d)
            win = const.tile([c.P, c.INT * c.H], F32)
            nc.sync.dma_start(out=win, in_=win_d)
            wl = const.tile([c.P, c.L * c.HT * c.H], F32)
            nc.sync.dma_start(out=wl, in_=wl_d)
            cin = const.tile([c.P, 3 * c.H], F32)     # b_in | g_in | beta_in bcast
            nc.sync.dma_start(out=cin, in_=cin_d)
            cl = const.tile([c.P, 3 * c.L * c.H], F32)  # bl | 0.9gl | 0.9betal bcast
            nc.sync.dma_start(out=cl, in_=cl_d)

            # ---- persistent state ----
            cur = state.tile([c.P, c.MT * c.H], F32)
            h0 = state.tile([c.P, c.MT * c.H], F32)
            curT = state.tile([c.P, c.HT * c.NPC], F32)
            mpart = state.tile([c.P, c.MT * c.H], BF16)
            # one m_full staging tile per AllGather part (phase-gated matmuls)
            mfull_p = [state.tile([c.P, c.NCORES * (e - s) * c.H], BF16,
                                  name=f"mfull{p}", tag=f"mfull{p}")
                       for p, (s, e) in enumerate(c.SPLITS)]

            H = c.H

            def rsqrt_dve(out, ve):
                """out = (ve)^-0.5 on DVE only: exponent-hack seed + 3 Newton
                iterations. ve is [128, n] f32, strictly positive."""
                n = ve.shape[-1]
                i32 = statp.tile([c.P, n], mybir.dt.int32, tag="rs_i")
                # i = ve_bits >> 1
                nc.vector.tensor_scalar(out=i32, in0=ve.bitcast(mybir.dt.int32),
                                        scalar1=1, scalar2=None,
                                        op0=ALU.logical_shift_right)
                # i = 0x5f3759df - i  ==  i * (-1) + 0x5f3759df
                nc.vector.tensor_scalar(out=i32, in0=i32, scalar1=-1,
                                        scalar2=0x5F3759DF, op0=ALU.mult,
                                        op1=ALU.add)
                y = statp.tile([c.P, n], F32, tag="rs_y")
                nc.vector.tensor_copy(out=y, in_=i32.bitcast(F32))
                w = statp.tile([c.P, n], F32, tag="rs_w")
                for _ in range(3):
                    # w = ve * y * y ; y = y * (1.5 - 0.5 w)
                    nc.vector.tensor_tensor(out=w, in0=y, in1=y, op=ALU.mult)
                    nc.vector.tensor_tensor(out=w, in0=w, in1=ve, op=ALU.mult)
                    nc.vector.tensor_scalar(out=w, in0=w, scalar1=-0.5,
                                            scalar2=1.5, op0=ALU.mult, op1=ALU.add)
                    nc.vector.tensor_tensor(out=y, in0=y, in1=w, op=ALU.mult)
                nc.vector.tensor_copy(out=out, in_=y)

            def epilogue(m, ps, bias, gain, beta, first):
                """LN(gelu(ps + bias)) * gain + beta; first block writes h0/cur,
                layers do the residual update. Also refreshes curT for tile m."""
                t1 = tmp.tile([c.P, H], F32, tag="t1")
                nc.vector.tensor_tensor(out=t1, in0=ps, in1=bias, op=ALU.add)
                t2 = tmp.tile([c.P, H], F32, tag="t2")
                nc.scalar.activation(out=t2, in_=t1, func=c.ACT)
                st6 = statp.tile([c.P, 6], F32, tag="st6")
                nc.vector.bn_stats(out=st6, in_=t2)
                mv = statp.tile([c.P, 2], F32, tag="mv")
                nc.vector.bn_aggr(out=mv, in_=st6)
                ve = statp.tile([c.P, 1], F32, tag="ve")
                nc.vector.tensor_scalar_add(ve, mv[:, 1:2], c.EPS)
                rinv = statp.tile([c.P, 1], F32, tag="rinv")
                rsqrt_dve(rinv, ve)
                # z = (t2 - mean) * rinv
                z = tmp.tile([c.P, H], F32, tag="z")
                nc.vector.tensor_scalar(out=z, in0=t2, scalar1=mv[:, 0:1],
                                        scalar2=rinv, op0=ALU.subtract,
                                        op1=ALU.mult)
                nc.vector.tensor_tensor(out=z, in0=z, in1=gain, op=ALU.mult)
                cs = cur[:, m * H:(m + 1) * H]
                if first:
                    h0s = h0[:, m * H:(m + 1) * H]
                    nc.vector.tensor_tensor(out=h0s, in0=z, in1=beta, op=ALU.add)
                    nc.vector.tensor_copy(out=cs, in_=h0s)
                else:
                    nc.vector.tensor_tensor(out=z, in0=z, in1=beta, op=ALU.add)
                    # base = 0.1*h0 + cur ; cur = z + base
                    base = tmp.tile([c.P, H], F32, tag="base")
                    nc.vector.scalar_tensor_tensor(
                        out=base, in0=h0[:, m * H:(m + 1) * H], scalar=c.ALPHA,
                        in1=cs, op0=ALU.mult, op1=ALU.add)
                    nc.vector.tensor_tensor(out=cs, in0=z, in1=base, op=ALU.add)

            def refresh_curT(m):
                for t in range(c.HT):
                    pt = tpp.tile([c.P, c.P], F32, tag="tp")
                    nc.tensor.transpose(
                        pt, cur[:, m * H + t * 128: m * H + t * 128 + 128], ident)
                    nc.vector.tensor_copy(
                        out=curT[:, t * c.NPC + m * 128: t * c.NPC + (m + 1) * 128],
                        in_=pt)

            # helpers for the pipelined schedule -----------------------
            def produce_m(lnext, m):
                """GEMM1 for layer lnext on tile m (fp32) + bf16 cast."""
                ps = g1p.tile([c.P, H], F32, tag="g1")
                for t in range(c.HT):
                    nc.tensor.matmul(
                        ps,
                        lhsT=curT[:, t * c.NPC + m * 128:
                                  t * c.NPC + (m + 1) * 128],
                        rhs=wl[:, (lnext * c.HT + t) * H:
                               (lnext * c.HT + t + 1) * H],
                        start=(t == 0), stop=(t == c.HT - 1))
                nc.vector.tensor_copy(out=mpart[:, m * H:(m + 1) * H], in_=ps)

            def emit_ag(l, p):
                s, e = c.SPLITS[p]
                nc.sync.dma_start(out=cc_in[l][p].ap(),
                                  in_=mpart[:, s * H:e * H])
                nc.gpsimd.collective_compute(
                    "AllGather", ALU.bypass, replica_groups=rg,
                    ins=[cc_in[l][p].ap()], outs=[cc_out[l][p].ap()])

            def emit_mfull(l):
                for p, (s, e) in enumerate(c.SPLITS):
                    w = (e - s) * c.H
                    for r in range(c.NCORES):
                        nc.sync.dma_start(
                            out=mfull_p[p][:, r * w:(r + 1) * w],
                            in_=cc_out[l][p].ap()[r * c.P:(r + 1) * c.P, :])

            # part-trigger chunk: AG part p of the NEXT layer fires once its
            # last m-tile's epilogue (and GEMM1) has been emitted
            trig = {}
            for p, (s, e) in enumerate(c.SPLITS):
                trig.setdefault((e - 1) // 2, []).append(p)

            # ================= input block =================
            for m in range(c.MT):
                ps = g1p.tile([c.P, H], F32, tag="g1")
                for t in range(c.INT):
                    nc.tensor.matmul(
                        ps,
                        lhsT=xT[:, t * c.NPC + m * 128: t * c.NPC + (m + 1) * 128],
                        rhs=win[:, t * H:(t + 1) * H],
                        start=(t == 0), stop=(t == c.INT - 1))
                epilogue(m, ps, cin[:, 0:H], cin[:, H:2 * H], cin[:, 2 * H:3 * H],
                         first=True)
                refresh_curT(m)
                produce_m(0, m)
                for p, (s, e) in enumerate(c.SPLITS):
                    if m == e - 1:
                        emit_ag(0, p)
            emit_mfull(0)

            # ================= layers =================
            def at_dma(ch):
                tiles = []
                for g in range(c.G):
                    t = atp.tile([c.P, c.KG * 256], BF16, tag="at")
                    nc.sync.dma_start(out=t, in_=At_d[ch, g])
                    tiles.append(t)
                return tiles

            for l in range(c.L):
                last = l == c.L - 1
                lb = cl[:, (3 * l) * H:(3 * l + 1) * H]
                lg = cl[:, (3 * l + 1) * H:(3 * l + 2) * H]
                lbeta = cl[:, (3 * l + 2) * H:(3 * l + 3) * H]
                at_tiles = {}
                for ch in range(c.MC):
                    if ch == 0:
                        at_tiles[0] = at_dma(0)
                        if c.MC > 1:
                            at_tiles[1] = at_dma(1)
                    elif ch + 1 < c.MC:
                        at_tiles[ch + 1] = at_dma(ch + 1)
                    atb = at_tiles.pop(ch)
                    ps0 = aggp.tile([c.P, H], F32, tag="agg")
                    ps1 = aggp.tile([c.P, H], F32, tag="agg")
                    for g in range(c.G):
                        for kk in range(c.KG):
                            j = g * c.KG + kk
                            p = c.KPART[j]
                            rhs = mfull_p[p][:, c.KOFF[j] * H:
                                             (c.KOFF[j] + 1) * H]
                            nc.tensor.matmul(
                                ps0, lhsT=atb[g][:, kk * 256: kk * 256 + 128],
                                rhs=rhs, start=(j == 0), stop=(j == c.KT - 1))
                            nc.tensor.matmul(
                                ps1, lhsT=atb[g][:, kk * 256 + 128:
                                                 kk * 256 + 256],
                                rhs=rhs, start=(j == 0), stop=(j == c.KT - 1))
                    for i, ps in ((0, ps0), (1, ps1)):
                        m = 2 * ch + i
                        epilogue(m, ps, lb, lg, lbeta, first=False)
                        if not last:
                            refresh_curT(m)
                            produce_m(l + 1, m)
                    if not last:
                        for p in trig.get(ch, []):
                            emit_ag(l + 1, p)
                if not last:
                    emit_mfull(l + 1)

            # ================= output =================
            out_v = out_d.rearrange("(m p) q -> p m q", p=c.P)
            nc.sync.dma_start(out=out_v, in_=cur.rearrange("p (m q) -> p m q", q=H))

    if split_waits:
        split_excess_waits(nc, wsplit_sem)
    return nc


# ---------------------------------------------------------- host wrapper

def prep_inputs(cfg, x, edge_index, W_in, b_in, g_in, beta_in, Wl, bl, gl, betal):
    """Build the per-core input maps (numpy, host-side)."""
    c = cfg
    x = np.asarray(x, dtype=np.float32)
    edge_index = np.asarray(edge_index)
    W_in = np.asarray(W_in, dtype=np.float32)
    b_in = np.asarray(b_in, dtype=np.float32)
    g_in = np.asarray(g_in, dtype=np.float32)
    beta_in = np.asarray(beta_in, dtype=np.float32)
    Wl = np.asarray(Wl, dtype=np.float32)
    bl = np.asarray(bl, dtype=np.float32)
    gl = np.asarray(gl, dtype=np.float32)
    betal = np.asarray(betal, dtype=np.float32)

    N = c.N
    # GCN normalization with self-loops
    src = np.concatenate([edge_index[0], np.arange(N, dtype=np.int64)])
    dst = np.concatenate([edge_index[1], np.arange(N, dtype=np.int64)])
    deg = np.bincount(dst, minlength=N).astype(np.float32)
    dinv = np.where(deg > 0, deg ** -0.5, 0.0).astype(np.float32)
    w = dinv[dst] * dinv[src]

    # padded global ids: node n -> core r = n // RPC, slot i = n % RPC
    nn = np.arange(N, dtype=np.int64)
    gid = (nn // c.RPC) * c.NPC + (nn % c.RPC)
    A = np.zeros((c.NPAD, c.NPAD), dtype=np.float32)
    np.add.at(A, (gid[dst], gid[src]), w)
    A16 = A.astype(ml_dtypes.bfloat16)
    del A

    # per-layer constants, broadcast across partitions
    def bcast(v):  # [H] -> [128, H]
        return np.broadcast_to(np.asarray(v, np.float32), (c.P, v.shape[-1])).copy()

    cin = np.concatenate([bcast(b_in), bcast(g_in), bcast(beta_in)], axis=1)
    cl_list = []
    for l in range(c.L):
        cl_list += [bcast(bl[l]), bcast((1.0 - c.ALPHA) * gl[l]),
                    bcast((1.0 - c.ALPHA) * betal[l])]
    cl_h = np.concatenate(cl_list, axis=1)

    # Wl host layout: [128, L*HT*H], k-tile t of layer l at cols (l*HT+t)*H
    wl_h = np.zeros((c.P, c.L * c.HT * c.H), np.float32)
    for l in range(c.L):
        for t in range(c.HT):
            wl_h[:, (l * c.HT + t) * c.H:(l * c.HT + t + 1) * c.H] = \
                Wl[l][t * 128:(t + 1) * 128, :]

    win_h = np.zeros((c.P, c.INT * c.H), np.float32)
    for t in range(c.INT):
        win_h[:, t * c.H:(t + 1) * c.H] = W_in[t * 128:(t + 1) * 128, :]

    in_maps = []
    for r in range(c.NCORES):
        lo, hi = r * c.RPC, min((r + 1) * c.RPC, N)
        xp = np.zeros((c.NPC, c.IN), np.float32)
        xp[:hi - lo] = x[lo:hi]
        xT = np.zeros((c.P, c.INT * c.NPC), np.float32)
        for t in range(c.INT):
            xT[:, t * c.NPC:(t + 1) * c.NPC] = xp[:, t * 128:(t + 1) * 128].T

        blk = A16[r * c.NPC:(r + 1) * c.NPC, :]          # [NPC, NPAD] (dst, src)
        t = np.ascontiguousarray(blk.T)                   # [NPAD, NPC] (src, dst)
        # permute k-tiles into the device iteration order (part-major), then
        # -> [MC, G, P, KG*256]: chunk ch covers dst cols ch*256..,
        # group g covers k-positions g*KG..
        t = t.reshape(c.KT, 128, c.NPC)[c.PERM]
        at = t.reshape(c.G, c.KG, 128, c.MC, 256)
        at = at.transpose(3, 0, 2, 1, 4).reshape(c.MC, c.G, 128, c.KG * 256)
        at = np.ascontiguousarray(at)

        in_maps.append({
            "xT": xT, "At": at, "Win": win_h, "Wlh": wl_h,
            "cin": cin, "cl": cl_h,
        })
    return in_maps


def postprocess(cfg, results):
    c = cfg
    out = np.empty((c.N, c.H), np.float32)
    for r in range(c.NCORES):
        lo, hi = r * c.RPC, min((r + 1) * c.RPC, c.N)
        out[lo:hi] = results[r]["out"][:hi - lo]
    return out


_CACHE = {}
TRACE = False  # test harness sets True to capture an NTFF profile


def kernel(x, edge_index, W_in, b_in, g_in, beta_in, Wl, bl, gl, betal):
    from concourse import bass_utils
    cfg = Cfg()
    in_maps = prep_inputs(cfg, x, edge_index, W_in, b_in, g_in, beta_in,
                          Wl, bl, gl, betal)
    if "nc" not in _CACHE:
        _CACHE["nc"] = build_nc(cfg)
    res = bass_utils.run_bass_kernel_spmd(
        _CACHE["nc"], in_maps, core_ids=list(range(cfg.NCORES)), trace=TRACE)
    _CACHE["last_result"] = res
    return postprocess(cfg, res.results)


# revision 24
# speedup vs baseline: 1.0604x; 1.0305x over previous
"""GCN encoder (6-layer) on 8 Trainium2 NeuronCores.

Strategy: the sparse GCN aggregation  agg = segment_sum(norm * m[src], dst)
is a fixed sparse-matrix product  agg = A @ m  with
A = D^-1/2 (Adj + I) D^-1/2  (10000x10000, ~0.33% dense, unstructured).
On the 128x128 PE array the dense formulation wins: nodes are sharded
1250 (padded to 1280) per core; each core streams its [10240 x 1280] A^T
shard (bf16, 26 MB) from HBM each layer while accumulating
agg_part = A_part @ m_full in PSUM (fp32).  m_full is rebuilt each layer
via a bf16 AllGather of the per-core GEMM1 results.  Epilogue
(bias + exact-erf GELU + LayerNorm + residual) runs on ACT/DVE, fully
overlapped with the PE stream.  LayerNorm rsqrt is computed on DVE
(Newton iterations over an exponent-hack seed) so the ACT table set
never leaves `gelu_and_others`.
"""

import math
import numpy as np
import ml_dtypes

import bass_rust
import concourse.bass as bass
import concourse.mybir as mybir
import concourse.tile as tile
from concourse.vector_clock import ScopedClock
from concourse.masks import make_identity

F32 = mybir.dt.float32
BF16 = mybir.dt.bfloat16
AF = mybir.ActivationFunctionType
ALU = mybir.AluOpType

# ---------------------------------------------------------------- config

class Cfg:
    def __init__(self, n_real=10000, mt=10, kg=16, l=6, h=256, in_dim=128,
                 alpha=0.1, eps=1e-5, ncores=8):
        self.P = 128
        self.NCORES = ncores
        self.MT = mt                      # m-tiles (128 rows) per core
        self.NPC = mt * 128               # padded nodes per core
        self.NPAD = self.NPC * ncores     # padded total nodes
        self.KT = self.NPAD // 128        # k-tiles in the big matmul
        self.KG = kg                      # k-tiles per A^T DMA group
        assert self.KT % (ncores * mt // ncores) == 0
        self.G = self.KT // kg            # DMA groups per chunk
        assert self.KT % kg == 0
        assert mt % 2 == 0
        self.MC = mt // 2                 # chunks of 2 m-tiles
        self.L = l
        self.H = h
        self.HT = h // 128                # h-tiles (2)
        self.IN = in_dim
        self.INT = in_dim // 128          # input k-tiles (1)
        self.N = n_real
        self.RPC = (n_real + ncores - 1) // ncores  # real rows per core
        assert self.RPC <= self.NPC
        self.ALPHA = alpha
        self.EPS = eps
        self.ACT = AF.Gelu  # sim test overrides (Gelu not implemented in sim)
        # chunks processed in pairs; each pair's tiles form one AllGather part
        self.PAIRS = []
        i = 0
        while i < self.MC:
            self.PAIRS.append(tuple(range(i, min(i + 2, self.MC))))
            i += 2
        self.SPLITS = [(pr[0] * 2, (pr[-1] + 1) * 2) for pr in self.PAIRS]
        for (s, e) in self.SPLITS:
            assert (ncores * (e - s)) % kg == 0, (s, e, kg)
        # global k-tile order of the big matmul: part-major, rank-major inside
        self.PERM = [r * mt + m
                     for (s, e) in self.SPLITS
                     for r in range(ncores)
                     for m in range(s, e)]
        # part index of each k-position
        self.KPART = []
        self.KOFF = []  # column offset (in k-tiles) inside that part's mfull
        off = 0
        for p, (s, e) in enumerate(self.SPLITS):
            n = ncores * (e - s)
            self.KPART += [p] * n
            self.KOFF += list(range(n))
            off += n


# ------------------------------------------------- drain-wait workaround

class SplitDrainTileContext(tile.TileContext):
    """This walrus build rejects >1 sync-wait on a CTRL (Drain) instruction;
    Tile's kernel-tail drain accumulates one wait per logical processor.
    Split the waits across a chain of drain instructions."""

    DRAIN_WAIT_CAP = 1

    def _drain_and_barrier(self, tick_clock, wait_clock):
        drain_inst = self.nc.sync.drain()
        wait_clock.add_sem_waits(
            drain_inst.ins, ScopedClock({None: tick_clock.global_clock})
        )
        si = drain_inst.ins.sync_info
        if si is not None:
            waits = list(si.on_wait)
            ups = list(si.on_update)
            cap = self.DRAIN_WAIT_CAP
            if len(waits) > cap:
                drain_inst.ins.sync_info = bass_rust.SyncInfo(
                    on_wait=waits[:cap], on_update=ups
                )
                rest = waits[cap:]
                for i in range(0, len(rest), cap):
                    d = self.nc.sync.drain()
                    d.ins.sync_info = bass_rust.SyncInfo(
                        on_wait=rest[i:i + cap], on_update=[]
                    )
        self.nc.all_engine_barrier()
        assert self.sems is not None
        popped = self.nc._tile_sem_poison_stack.pop()
        assert popped is self._sem_poison
        self.nc.clear_and_free_semaphores(list(self.sems.allocated().values()))
        self.nc.all_engine_barrier()


# This walrus build caps sync-waits at 1 per instruction. Tile packs one wait
# per producer proc onto consumer instructions. Rewrite:
#  - engine-executed instructions: move excess waits onto same-engine NoOps
#    inserted just before the instruction (engine subsequence order preserved)
#  - DMACopy (queue-executed -- a NoOp cannot sit in a DGE queue): move ALL its
#    waits onto an SP NoOp chain whose last link bumps a helper semaphore; the
#    DMA then waits only `helper >= k`. Safe because every producer of the
#    moved waits is scheduled before this program point, so blocking SP here
#    cannot deadlock (SP has already pushed all earlier descriptors).
_SEM_CHAIN_OPCODES = {"DMACopy", "TriggerCollective", "CollectiveCompute"}


def split_excess_waits(nc, helper, cap=1):
    fn = nc.m.functions[0]
    ctr = 0
    kval = 0
    sp = mybir.EngineType.SP
    used_helper = False
    for bb in fn.blocks:
        out = []
        changed = False
        for inst in bb.instructions:
            si = inst.sync_info
            n_w = len(si.on_wait) if si is not None else 0
            if n_w > cap and inst.opcode not in _SEM_CHAIN_OPCODES:
                waits = list(si.on_wait)
                extra = waits[cap:]
                for j in range(0, len(extra), cap):
                    ctr += 1
                    n = bass_rust.InstNoOp(name=f"wsplit-{ctr}", ins=[], outs=[])
                    n.engine = inst.engine
                    n.bass_nofuse = True
                    n.sync_info = bass_rust.SyncInfo(
                        on_wait=extra[j:j + cap], on_update=[])
                    out.append(n)
                inst.sync_info = bass_rust.SyncInfo(
                    on_wait=waits[:cap], on_update=list(si.on_update))
                changed = True
            elif n_w > cap:
                # queue-executed: SP NoOp chain, one wait each; last bumps the
                # helper; the instruction waits helper >= kval only.
                waits = list(si.on_wait)
                kval += 1
                used_helper = True
                for j, w in enumerate(waits):
                    ctr += 1
                    n = bass_rust.InstNoOp(name=f"wsplit-{ctr}", ins=[], outs=[])
                    n.engine = sp
                    n.bass_nofuse = True
                    ups = []
                    if j == len(waits) - 1:
                        ups = [bass_rust.SyncUpdate(
                            ant_name=helper.name, id=helper.num,
                            sync_type="semaphore", update_mode="sem-inc",
                            update_value=1)]
                    n.sync_info = bass_rust.SyncInfo(on_wait=[w], on_update=ups)
                    out.append(n)
                hw = bass_rust.SyncWait(
                    ant_name=helper.name, id=helper.num, sync_type="semaphore",
                    wait_mode="sem-ge-imm", wait_value=kval)
                inst.sync_info = bass_rust.SyncInfo(
                    on_wait=[hw], on_update=list(si.on_update))
                changed = True
            out.append(inst)
        if changed:
            bb.instructions = out
    if used_helper:
        # reset for any later execution of the NEFF (NRT does not zero kernel
        # semaphores between executions; Tile clears only its own)
        nc.sync.sem_clear(helper)
    return ctr


# ---------------------------------------------------------- device kernel

def build_nc(cfg: Cfg, split_waits=True):
    c = cfg
    nc = bass.Bass("TRN2", target_bir_lowering=False, debug=False,
                   num_devices=c.NCORES)
    # reserved before TileContext so Tile can never hand out the same sem id
    wsplit_sem = nc.alloc_semaphore("wsplit_dma") if split_waits else None

    # ---- I/O ----
    xT_d = nc.dram_tensor("xT", [c.P, c.INT * c.NPC], F32, kind="ExternalInput").ap()
    At_d = nc.dram_tensor("At", [c.MC, c.G, c.P, c.KG * 256], BF16,
                          kind="ExternalInput").ap()
    win_d = nc.dram_tensor("Win", [c.P, c.INT * c.H], F32, kind="ExternalInput").ap()
    wl_d = nc.dram_tensor("Wlh", [c.P, c.L * c.HT * c.H], F32,
                          kind="ExternalInput").ap()
    cin_d = nc.dram_tensor("cin", [c.P, 3 * c.H], F32, kind="ExternalInput").ap()
    cl_d = nc.dram_tensor("cl", [c.P, 3 * c.L * c.H], F32, kind="ExternalInput").ap()
    out_d = nc.dram_tensor("out", [c.NPC, c.H], F32, kind="ExternalOutput").ap()

    # collective bounce buffers (per layer, per AllGather part)
    nparts = len(c.SPLITS)
    cc_in = [[nc.dram_tensor(f"cc_in_{l}_{p}", [c.P, (e - s) * c.H], BF16)
              for p, (s, e) in enumerate(c.SPLITS)] for l in range(c.L)]
    cc_out = [[nc.dram_tensor(f"cc_out_{l}_{p}",
                              [c.P * c.NCORES, (e - s) * c.H], BF16,
                              addr_space="Shared")
               for p, (s, e) in enumerate(c.SPLITS)] for l in range(c.L)]
    rg = [list(range(c.NCORES))]

    with SplitDrainTileContext(nc) as tc:
        with (
            tc.tile_pool(name="const", bufs=1) as const,
            tc.tile_pool(name="state", bufs=1) as state,
            tc.tile_pool(name="at", bufs=3) as atp,
            tc.tile_pool(name="tmp", bufs=4) as tmp,
            tc.tile_pool(name="stat", bufs=4) as statp,
            tc.tile_pool(name="agg", bufs=6, space="PSUM") as aggp,
            tc.tile_pool(name="g1", bufs=1, space="PSUM") as g1p,
            tc.tile_pool(name="tp", bufs=1, space="PSUM") as tpp,
        ):
            # ---- constants ----
            ident = const.tile([c.P, c.P], F32)
            make_identity(nc, ident)
            xT = const.tile([c.P, c.INT * c.NPC], F32)
            nc.sync.dma_start(out=xT, in_=xT_d)
            win = const.tile([c.P, c.INT * c.H], F32)
            nc.sync.dma_start(out=win, in_=win_d)
            wl = const.tile([c.P, c.L * c.HT * c.H], F32)
            nc.sync.dma_start(out=wl, in_=wl_d)
            cin = const.tile([c.P, 3 * c.H], F32)     # b_in | g_in | beta_in bcast
            nc.sync.dma_start(out=cin, in_=cin_d)
            cl = const.tile([c.P, 3 * c.L * c.H], F32)  # bl | 0.9gl | 0.9betal bcast
            nc.sync.dma_start(out=cl, in_=cl_d)

            # ---- persistent state ----
            cur = state.tile([c.P, c.MT * c.H], F32)
            h0 = state.tile([c.P, c.MT * c.H], F32)
            curT = state.tile([c.P, c.HT * c.NPC], F32)
            mpart = state.tile([c.P, c.MT * c.H], BF16)
            # one m_full staging tile per AllGather part (phase-gated matmuls)
            mfull_p = [state.tile([c.P, c.NCORES * (e - s) * c.H], BF16,
                                  name=f"mfull{p}", tag=f"mfull{p}")
                       for p, (s, e) in enumerate(c.SPLITS)]

            H = c.H

            def rsqrt_dve(out, ve):
                """out = (ve)^-0.5 on DVE only: exponent-hack seed + 3 Newton
                iterations. ve is [128, n] f32, strictly positive."""
                n = ve.shape[-1]
                i32 = statp.tile([c.P, n], mybir.dt.int32, tag="rs_i")
                # i = ve_bits >> 1
                nc.vector.tensor_scalar(out=i32, in0=ve.bitcast(mybir.dt.int32),
                                        scalar1=1, scalar2=None,
                                        op0=ALU.logical_shift_right)
                # i = 0x5f3759df - i  ==  i * (-1) + 0x5f3759df
                nc.vector.tensor_scalar(out=i32, in0=i32, scalar1=-1,
                                        scalar2=0x5F3759DF, op0=ALU.mult,
                                        op1=ALU.add)
                y = statp.tile([c.P, n], F32, tag="rs_y")
                nc.vector.tensor_copy(out=y, in_=i32.bitcast(F32))
                w = statp.tile([c.P, n], F32, tag="rs_w")
                for _ in range(3):
                    # w = ve * y * y ; y = y * (1.5 - 0.5 w)
                    nc.vector.tensor_tensor(out=w, in0=y, in1=y, op=ALU.mult)
                    nc.vector.tensor_tensor(out=w, in0=w, in1=ve, op=ALU.mult)
                    nc.vector.tensor_scalar(out=w, in0=w, scalar1=-0.5,
                                            scalar2=1.5, op0=ALU.mult, op1=ALU.add)
                    nc.vector.tensor_tensor(out=y, in0=y, in1=w, op=ALU.mult)
                nc.vector.tensor_copy(out=out, in_=y)

            def epilogue(m, ps, bias, gain, beta, first):
                """LN(gelu(ps + bias)) * gain + beta; first block writes h0/cur,
                layers do the residual update. Also refreshes curT for tile m."""
                t1 = tmp.tile([c.P, H], F32, tag="t1")
                nc.vector.tensor_tensor(out=t1, in0=ps, in1=bias, op=ALU.add)
                t2 = tmp.tile([c.P, H], F32, tag="t2")
                nc.scalar.activation(out=t2, in_=t1, func=c.ACT)
                st6 = statp.tile([c.P, 6], F32, tag="st6")
                nc.vector.bn_stats(out=st6, in_=t2)
                mv = statp.tile([c.P, 2], F32, tag="mv")
                nc.vector.bn_aggr(out=mv, in_=st6)
                ve = statp.tile([c.P, 1], F32, tag="ve")
                nc.vector.tensor_scalar_add(ve, mv[:, 1:2], c.EPS)
                rinv = statp.tile([c.P, 1], F32, tag="rinv")
                rsqrt_dve(rinv, ve)
                # z = (t2 - mean) * rinv
                z = tmp.tile([c.P, H], F32, tag="z")
                nc.vector.tensor_scalar(out=z, in0=t2, scalar1=mv[:, 0:1],
                                        scalar2=rinv, op0=ALU.subtract,
                                        op1=ALU.mult)
                nc.vector.tensor_tensor(out=z, in0=z, in1=gain, op=ALU.mult)
                cs = cur[:, m * H:(m + 1) * H]
                if first:
                    h0s = h0[:, m * H:(m + 1) * H]
                    nc.vector.tensor_tensor(out=h0s, in0=z, in1=beta, op=ALU.add)
                    nc.vector.tensor_copy(out=cs, in_=h0s)
                else:
                    nc.vector.tensor_tensor(out=z, in0=z, in1=beta, op=ALU.add)
                    # base = 0.1*h0 + cur ; cur = z + base
                    base = tmp.tile([c.P, H], F32, tag="base")
                    nc.vector.scalar_tensor_tensor(
                        out=base, in0=h0[:, m * H:(m + 1) * H], scalar=c.ALPHA,
                        in1=cs, op0=ALU.mult, op1=ALU.add)
                    nc.vector.tensor_tensor(out=cs, in0=z, in1=base, op=ALU.add)

            def refresh_curT(m):
                for t in range(c.HT):
                    pt = tpp.tile([c.P, c.P], F32, tag="tp")
                    nc.tensor.transpose(
                        pt, cur[:, m * H + t * 128: m * H + t * 128 + 128], ident)
                    nc.vector.tensor_copy(
                        out=curT[:, t * c.NPC + m * 128: t * c.NPC + (m + 1) * 128],
                        in_=pt)

            # helpers for the pipelined schedule -----------------------
            nparts = len(c.SPLITS)
            # phase p covers k-positions [poff[p], poff[p+1])
            poff = [0]
            for (s, e) in c.SPLITS:
                poff.append(poff[-1] + c.NCORES * (e - s))

            def produce_m(lnext, m):
                """GEMM1 for layer lnext on tile m (fp32) + bf16 cast."""
                ps = g1p.tile([c.P, H], F32, tag="g1")
                for t in range(c.HT):
                    nc.tensor.matmul(
                        ps,
                        lhsT=curT[:, t * c.NPC + m * 128:
                                  t * c.NPC + (m + 1) * 128],
                        rhs=wl[:, (lnext * c.HT + t) * H:
                               (lnext * c.HT + t + 1) * H],
                        start=(t == 0), stop=(t == c.HT - 1))
                nc.vector.tensor_copy(out=mpart[:, m * H:(m + 1) * H], in_=ps)

            def emit_ag(l, p):
                s, e = c.SPLITS[p]
                nc.sync.dma_start(out=cc_in[l][p].ap(),
                                  in_=mpart[:, s * H:e * H])
                nc.gpsimd.collective_compute(
                    "AllGather", ALU.bypass, replica_groups=rg,
                    ins=[cc_in[l][p].ap()], outs=[cc_out[l][p].ap()])

            def emit_mfull(l, p):
                s, e = c.SPLITS[p]
                w = (e - s) * c.H
                for r in range(c.NCORES):
                    nc.sync.dma_start(
                        out=mfull_p[p][:, r * w:(r + 1) * w],
                        in_=cc_out[l][p].ap()[r * c.P:(r + 1) * c.P, :])

            def tile_tail(l, m):
                """transpose + next-layer GEMM1 for tile m of layer l."""
                refresh_curT(m)
                produce_m(l + 1, m)

            def at_dma(ch, grp):
                t = atp.tile([c.P, c.KG * 256], BF16, tag="at")
                nc.sync.dma_start(out=t, in_=At_d[ch, grp])
                return t

            def phase_groups(p):
                return range(poff[p] // c.KG, poff[p + 1] // c.KG)

            # accumulators: both m-tiles of a chunk share one PSUM bank
            at_tiles = {}

            def mm_phase(pair, p, acc, atg):
                for ch in pair:
                    for g in phase_groups(p):
                        atb = atg.pop((ch, g))
                        for kk in range(c.KG):
                            j = g * c.KG + kk
                            rhs = mfull_p[c.KPART[j]][
                                :, c.KOFF[j] * H:(c.KOFF[j] + 1) * H]
                            nc.tensor.matmul(
                                acc[ch][0],
                                lhsT=atb[:, kk * 256: kk * 256 + 128],
                                rhs=rhs, start=(j == 0), stop=(j == c.KT - 1))
                            nc.tensor.matmul(
                                acc[ch][1],
                                lhsT=atb[:, kk * 256 + 128: kk * 256 + 256],
                                rhs=rhs, start=(j == 0), stop=(j == c.KT - 1))

            # ================= input block =================
            instage = []
            for m in range(c.MT):
                ps = g1p.tile([c.P, H], F32, tag="g1")
                for t in range(c.INT):
                    nc.tensor.matmul(
                        ps,
                        lhsT=xT[:, t * c.NPC + m * 128: t * c.NPC + (m + 1) * 128],
                        rhs=win[:, t * H:(t + 1) * H],
                        start=(t == 0), stop=(t == c.INT - 1))
                st = tmp.tile([c.P, H], F32, tag="instage", bufs=c.MT,
                              name=f"instage{m}")
                nc.vector.tensor_copy(out=st, in_=ps)
                instage.append(st)
            for p, (s, e) in enumerate(c.SPLITS):
                for m in range(s, e):
                    epilogue(m, instage[m], cin[:, 0:H], cin[:, H:2 * H],
                             cin[:, 2 * H:3 * H], first=True)
                    tile_tail(-1, m)   # produce_m uses lnext = 0
                emit_ag(0, p)
                emit_mfull(0, p)

            # ================= layers =================
            # Tail work (transpose+GEMM1+AG+mfull) is deferred so the PE never
            # waits on a DVE epilogue: each pair's tails are emitted between
            # the NEXT pair's matmul phases; the last pair's tails (pending)
            # land inside the next layer. mfull DMA placement avoids blocking
            # SP ahead of A^T prefetch pushes (see split_excess_waits).
            pending = None
            for l in range(c.L):
                last = l == c.L - 1
                lb = cl[:, (3 * l) * H:(3 * l + 1) * H]
                lg = cl[:, (3 * l + 1) * H:(3 * l + 2) * H]
                lbeta = cl[:, (3 * l + 2) * H:(3 * l + 3) * H]
                acc = {}

                def close_pair(pair):
                    for ch in pair:
                        for i in (0, 1):
                            m = 2 * ch + i
                            epilogue(m, acc[ch][i], lb, lg, lbeta,
                                     first=False)

                for pi, pair in enumerate(c.PAIRS):
                    for ch in pair:
                        acc[ch] = (aggp.tile([c.P, H], F32, tag="agg",
                                             name=f"acc_l{l}_c{ch}_0"),
                                   aggp.tile([c.P, H], F32, tag="agg",
                                             name=f"acc_l{l}_c{ch}_1"))
                    for p in range(nparts):
                        for ch in pair:
                            for g in phase_groups(p):
                                at_tiles[(ch, g)] = at_dma(ch, g)
                        if pi == 0 and pending is not None:
                            # deferred mfull parts for THIS layer's m (emitted
                            # after this phase's At pushes so SP blocking on
                            # the AG can't starve the A^T prefetch)
                            pl = pending[0]
                            if p == 0 and nparts >= 3:
                                emit_mfull(pl + 1, 1)
                            if p == min(1, nparts - 1):
                                for m in pending[1]:
                                    tile_tail(pl, m)
                                emit_ag(pl + 1, nparts - 1)
                                emit_mfull(pl + 1, nparts - 1)
                                pending = None
                        mm_phase(pair, p, acc, at_tiles)
                        if pi > 0 and p < len(c.PAIRS[pi - 1]) and not last:
                            chp = c.PAIRS[pi - 1][p]
                            for m in (2 * chp, 2 * chp + 1):
                                tile_tail(l, m)
                            if chp == c.PAIRS[pi - 1][-1] and len(c.PAIRS) > 1:
                                emit_ag(l + 1, pi - 1)
                    close_pair(pair)
                if not last:
                    lastpair = c.PAIRS[-1]
                    pending = (l, [m for ch in lastpair
                                   for m in (2 * ch, 2 * ch + 1)])
                    if nparts >= 2:
                        # part 0 feeds the next layer's first matmul phase;
                        # its WAR (vs this layer's phase-0 readers) has
                        # cleared by now, and its AG fired mid-layer
                        emit_mfull(l + 1, 0)

            # ================= output =================
            out_v = out_d.rearrange("(m p) q -> p m q", p=c.P)
            nc.sync.dma_start(out=out_v, in_=cur.rearrange("p (m q) -> p m q", q=H))

    if split_waits:
        split_excess_waits(nc, wsplit_sem)
    return nc


# ---------------------------------------------------------- host wrapper

def prep_inputs(cfg, x, edge_index, W_in, b_in, g_in, beta_in, Wl, bl, gl, betal):
    """Build the per-core input maps (numpy, host-side)."""
    c = cfg
    x = np.asarray(x, dtype=np.float32)
    edge_index = np.asarray(edge_index)
    W_in = np.asarray(W_in, dtype=np.float32)
    b_in = np.asarray(b_in, dtype=np.float32)
    g_in = np.asarray(g_in, dtype=np.float32)
    beta_in = np.asarray(beta_in, dtype=np.float32)
    Wl = np.asarray(Wl, dtype=np.float32)
    bl = np.asarray(bl, dtype=np.float32)
    gl = np.asarray(gl, dtype=np.float32)
    betal = np.asarray(betal, dtype=np.float32)

    N = c.N
    # GCN normalization with self-loops
    src = np.concatenate([edge_index[0], np.arange(N, dtype=np.int64)])
    dst = np.concatenate([edge_index[1], np.arange(N, dtype=np.int64)])
    deg = np.bincount(dst, minlength=N).astype(np.float32)
    dinv = np.where(deg > 0, deg ** -0.5, 0.0).astype(np.float32)
    w = dinv[dst] * dinv[src]

    # padded global ids: node n -> core r = n // RPC, slot i = n % RPC
    nn = np.arange(N, dtype=np.int64)
    gid = (nn // c.RPC) * c.NPC + (nn % c.RPC)
    A = np.zeros((c.NPAD, c.NPAD), dtype=np.float32)
    np.add.at(A, (gid[dst], gid[src]), w)
    A16 = A.astype(ml_dtypes.bfloat16)
    del A

    # per-layer constants, broadcast across partitions
    def bcast(v):  # [H] -> [128, H]
        return np.broadcast_to(np.asarray(v, np.float32), (c.P, v.shape[-1])).copy()

    cin = np.concatenate([bcast(b_in), bcast(g_in), bcast(beta_in)], axis=1)
    cl_list = []
    for l in range(c.L):
        cl_list += [bcast(bl[l]), bcast((1.0 - c.ALPHA) * gl[l]),
                    bcast((1.0 - c.ALPHA) * betal[l])]
    cl_h = np.concatenate(cl_list, axis=1)

    # Wl host layout: [128, L*HT*H], k-tile t of layer l at cols (l*HT+t)*H
    wl_h = np.zeros((c.P, c.L * c.HT * c.H), np.float32)
    for l in range(c.L):
        for t in range(c.HT):
            wl_h[:, (l * c.HT + t) * c.H:(l * c.HT + t + 1) * c.H] = \
                Wl[l][t * 128:(t + 1) * 128, :]

    win_h = np.zeros((c.P, c.INT * c.H), np.float32)
    for t in range(c.INT):
        win_h[:, t * c.H:(t + 1) * c.H] = W_in[t * 128:(t + 1) * 128, :]

    in_maps = []
    for r in range(c.NCORES):
        lo, hi = r * c.RPC, min((r + 1) * c.RPC, N)
        xp = np.zeros((c.NPC, c.IN), np.float32)
        xp[:hi - lo] = x[lo:hi]
        xT = np.zeros((c.P, c.INT * c.NPC), np.float32)
        for t in range(c.INT):
            xT[:, t * c.NPC:(t + 1) * c.NPC] = xp[:, t * 128:(t + 1) * 128].T

        blk = A16[r * c.NPC:(r + 1) * c.NPC, :]          # [NPC, NPAD] (dst, src)
        t = np.ascontiguousarray(blk.T)                   # [NPAD, NPC] (src, dst)
        # permute k-tiles into the device iteration order (part-major), then
        # -> [MC, G, P, KG*256]: chunk ch covers dst cols ch*256..,
        # group g covers k-positions g*KG..
        t = t.reshape(c.KT, 128, c.NPC)[c.PERM]
        at = t.reshape(c.G, c.KG, 128, c.MC, 256)
        at = at.transpose(3, 0, 2, 1, 4).reshape(c.MC, c.G, 128, c.KG * 256)
        at = np.ascontiguousarray(at)

        in_maps.append({
            "xT": xT, "At": at, "Win": win_h, "Wlh": wl_h,
            "cin": cin, "cl": cl_h,
        })
    return in_maps


def postprocess(cfg, results):
    c = cfg
    out = np.empty((c.N, c.H), np.float32)
    for r in range(c.NCORES):
        lo, hi = r * c.RPC, min((r + 1) * c.RPC, c.N)
        out[lo:hi] = results[r]["out"][:hi - lo]
    return out


_CACHE = {}
TRACE = False  # test harness sets True to capture an NTFF profile


def kernel(x, edge_index, W_in, b_in, g_in, beta_in, Wl, bl, gl, betal):
    from concourse import bass_utils
    cfg = Cfg()
    in_maps = prep_inputs(cfg, x, edge_index, W_in, b_in, g_in, beta_in,
                          Wl, bl, gl, betal)
    if "nc" not in _CACHE:
        _CACHE["nc"] = build_nc(cfg)
    res = bass_utils.run_bass_kernel_spmd(
        _CACHE["nc"], in_maps, core_ids=list(range(cfg.NCORES)), trace=TRACE)
    _CACHE["last_result"] = res
    return postprocess(cfg, res.results)


# revision 26
# speedup vs baseline: 1.0818x; 1.0202x over previous
"""GCN encoder (6-layer) on 8 Trainium2 NeuronCores.

Strategy: the sparse GCN aggregation  agg = segment_sum(norm * m[src], dst)
is a fixed sparse-matrix product  agg = A @ m  with
A = D^-1/2 (Adj + I) D^-1/2  (10000x10000, ~0.33% dense, unstructured).
On the 128x128 PE array the dense formulation wins: nodes are sharded
1250 (padded to 1280) per core; each core streams its [10240 x 1280] A^T
shard (bf16, 26 MB) from HBM each layer while accumulating
agg_part = A_part @ m_full in PSUM (fp32).  m_full is rebuilt each layer
via a bf16 AllGather of the per-core GEMM1 results.  Epilogue
(bias + exact-erf GELU + LayerNorm + residual) runs on ACT/DVE, fully
overlapped with the PE stream.  LayerNorm rsqrt is computed on DVE
(Newton iterations over an exponent-hack seed) so the ACT table set
never leaves `gelu_and_others`.
"""

import math
import numpy as np
import ml_dtypes

import bass_rust
import concourse.bass as bass
import concourse.mybir as mybir
import concourse.tile as tile
from concourse.vector_clock import ScopedClock
from concourse.masks import make_identity

F32 = mybir.dt.float32
F32R = mybir.dt.float32r
BF16 = mybir.dt.bfloat16
AF = mybir.ActivationFunctionType
ALU = mybir.AluOpType

# ---------------------------------------------------------------- config

class Cfg:
    def __init__(self, n_real=10000, mt=10, kg=16, l=6, h=256, in_dim=128,
                 alpha=0.1, eps=1e-5, ncores=8):
        self.P = 128
        self.NCORES = ncores
        self.MT = mt                      # m-tiles (128 rows) per core
        self.NPC = mt * 128               # padded nodes per core
        self.NPAD = self.NPC * ncores     # padded total nodes
        self.KT = self.NPAD // 128        # k-tiles in the big matmul
        self.KG = kg                      # k-tiles per A^T DMA group
        assert self.KT % (ncores * mt // ncores) == 0
        self.G = self.KT // kg            # DMA groups per chunk
        assert self.KT % kg == 0
        assert mt % 2 == 0
        self.MC = mt // 2                 # chunks of 2 m-tiles
        self.L = l
        self.H = h
        self.HT = h // 128                # h-tiles (2)
        self.IN = in_dim
        self.INT = in_dim // 128          # input k-tiles (1)
        self.N = n_real
        self.RPC = (n_real + ncores - 1) // ncores  # real rows per core
        assert self.RPC <= self.NPC
        self.ALPHA = alpha
        self.EPS = eps
        self.ACT = AF.Gelu  # sim test overrides (Gelu not implemented in sim)
        # chunks processed in pairs; each pair's tiles form one AllGather part
        self.PAIRS = []
        i = 0
        while i < self.MC:
            self.PAIRS.append(tuple(range(i, min(i + 2, self.MC))))
            i += 2
        self.SPLITS = [(pr[0] * 2, (pr[-1] + 1) * 2) for pr in self.PAIRS]
        for (s, e) in self.SPLITS:
            assert (ncores * (e - s)) % kg == 0, (s, e, kg)
        # global k-tile order of the big matmul: part-major, rank-major inside
        self.PERM = [r * mt + m
                     for (s, e) in self.SPLITS
                     for r in range(ncores)
                     for m in range(s, e)]
        # part index of each k-position
        self.KPART = []
        self.KOFF = []  # column offset (in k-tiles) inside that part's mfull
        off = 0
        for p, (s, e) in enumerate(self.SPLITS):
            n = ncores * (e - s)
            self.KPART += [p] * n
            self.KOFF += list(range(n))
            off += n


# ------------------------------------------------- drain-wait workaround

class SplitDrainTileContext(tile.TileContext):
    """This walrus build rejects >1 sync-wait on a CTRL (Drain) instruction;
    Tile's kernel-tail drain accumulates one wait per logical processor.
    Split the waits across a chain of drain instructions."""

    DRAIN_WAIT_CAP = 1

    def _drain_and_barrier(self, tick_clock, wait_clock):
        drain_inst = self.nc.sync.drain()
        wait_clock.add_sem_waits(
            drain_inst.ins, ScopedClock({None: tick_clock.global_clock})
        )
        si = drain_inst.ins.sync_info
        if si is not None:
            waits = list(si.on_wait)
            ups = list(si.on_update)
            cap = self.DRAIN_WAIT_CAP
            if len(waits) > cap:
                drain_inst.ins.sync_info = bass_rust.SyncInfo(
                    on_wait=waits[:cap], on_update=ups
                )
                rest = waits[cap:]
                for i in range(0, len(rest), cap):
                    d = self.nc.sync.drain()
                    d.ins.sync_info = bass_rust.SyncInfo(
                        on_wait=rest[i:i + cap], on_update=[]
                    )
        self.nc.all_engine_barrier()
        assert self.sems is not None
        popped = self.nc._tile_sem_poison_stack.pop()
        assert popped is self._sem_poison
        self.nc.clear_and_free_semaphores(list(self.sems.allocated().values()))
        self.nc.all_engine_barrier()


# This walrus build caps sync-waits at 1 per instruction. Tile packs one wait
# per producer proc onto consumer instructions. Rewrite:
#  - engine-executed instructions: move excess waits onto same-engine NoOps
#    inserted just before the instruction (engine subsequence order preserved)
#  - DMACopy (queue-executed -- a NoOp cannot sit in a DGE queue): move ALL its
#    waits onto an SP NoOp chain whose last link bumps a helper semaphore; the
#    DMA then waits only `helper >= k`. Safe because every producer of the
#    moved waits is scheduled before this program point, so blocking SP here
#    cannot deadlock (SP has already pushed all earlier descriptors).
_SEM_CHAIN_OPCODES = {"DMACopy", "TriggerCollective", "CollectiveCompute"}


def split_excess_waits(nc, helper, cap=1):
    fn = nc.m.functions[0]
    ctr = 0
    kval = 0
    sp = mybir.EngineType.SP
    used_helper = False
    for bb in fn.blocks:
        out = []
        changed = False
        for inst in bb.instructions:
            si = inst.sync_info
            n_w = len(si.on_wait) if si is not None else 0
            if n_w > cap and inst.opcode not in _SEM_CHAIN_OPCODES:
                waits = list(si.on_wait)
                extra = waits[cap:]
                for j in range(0, len(extra), cap):
                    ctr += 1
                    n = bass_rust.InstNoOp(name=f"wsplit-{ctr}", ins=[], outs=[])
                    n.engine = inst.engine
                    n.bass_nofuse = True
                    n.sync_info = bass_rust.SyncInfo(
                        on_wait=extra[j:j + cap], on_update=[])
                    out.append(n)
                inst.sync_info = bass_rust.SyncInfo(
                    on_wait=waits[:cap], on_update=list(si.on_update))
                changed = True
            elif n_w > cap:
                # queue-executed: SP NoOp chain, one wait each; last bumps the
                # helper; the instruction waits helper >= kval only.
                waits = list(si.on_wait)
                kval += 1
                used_helper = True
                for j, w in enumerate(waits):
                    ctr += 1
                    n = bass_rust.InstNoOp(name=f"wsplit-{ctr}", ins=[], outs=[])
                    n.engine = sp
                    n.bass_nofuse = True
                    ups = []
                    if j == len(waits) - 1:
                        ups = [bass_rust.SyncUpdate(
                            ant_name=helper.name, id=helper.num,
                            sync_type="semaphore", update_mode="sem-inc",
                            update_value=1)]
                    n.sync_info = bass_rust.SyncInfo(on_wait=[w], on_update=ups)
                    out.append(n)
                hw = bass_rust.SyncWait(
                    ant_name=helper.name, id=helper.num, sync_type="semaphore",
                    wait_mode="sem-ge-imm", wait_value=kval)
                inst.sync_info = bass_rust.SyncInfo(
                    on_wait=[hw], on_update=list(si.on_update))
                changed = True
            out.append(inst)
        if changed:
            bb.instructions = out
    if used_helper:
        # reset for any later execution of the NEFF (NRT does not zero kernel
        # semaphores between executions; Tile clears only its own)
        nc.sync.sem_clear(helper)
    return ctr


# ---------------------------------------------------------- device kernel

def build_nc(cfg: Cfg, split_waits=True):
    c = cfg
    nc = bass.Bass("TRN2", target_bir_lowering=False, debug=False,
                   num_devices=c.NCORES)
    # reserved before TileContext so Tile can never hand out the same sem id
    wsplit_sem = nc.alloc_semaphore("wsplit_dma") if split_waits else None

    # ---- I/O ----
    xT_d = nc.dram_tensor("xT", [c.P, c.INT * c.NPC], F32R, kind="ExternalInput").ap()
    At_d = nc.dram_tensor("At", [c.MC, c.G, c.P, c.KG * 256], BF16,
                          kind="ExternalInput").ap()
    win_d = nc.dram_tensor("Win", [c.P, c.INT * c.H], F32R, kind="ExternalInput").ap()
    wl_d = nc.dram_tensor("Wlh", [c.P, c.L * c.HT * c.H], F32R,
                          kind="ExternalInput").ap()
    cin_d = nc.dram_tensor("cin", [c.P, 3 * c.H], F32, kind="ExternalInput").ap()
    cl_d = nc.dram_tensor("cl", [c.P, 3 * c.L * c.H], F32, kind="ExternalInput").ap()
    out_d = nc.dram_tensor("out", [c.NPC, c.H], F32, kind="ExternalOutput").ap()

    # collective bounce buffers (per layer, per AllGather part)
    nparts = len(c.SPLITS)
    cc_in = [[nc.dram_tensor(f"cc_in_{l}_{p}", [c.P, (e - s) * c.H], BF16)
              for p, (s, e) in enumerate(c.SPLITS)] for l in range(c.L)]
    cc_out = [[nc.dram_tensor(f"cc_out_{l}_{p}",
                              [c.P * c.NCORES, (e - s) * c.H], BF16,
                              addr_space="Shared")
               for p, (s, e) in enumerate(c.SPLITS)] for l in range(c.L)]
    rg = [list(range(c.NCORES))]

    with SplitDrainTileContext(nc) as tc:
        with (
            tc.tile_pool(name="const", bufs=1) as const,
            tc.tile_pool(name="state", bufs=1) as state,
            tc.tile_pool(name="at", bufs=3) as atp,
            tc.tile_pool(name="tmp", bufs=4) as tmp,
            tc.tile_pool(name="stat", bufs=4) as statp,
            tc.tile_pool(name="agg", bufs=6, space="PSUM") as aggp,
            tc.tile_pool(name="g1", bufs=1, space="PSUM") as g1p,
            tc.tile_pool(name="tp", bufs=1, space="PSUM") as tpp,
        ):
            # ---- constants ----
            ident = const.tile([c.P, c.P], F32)
            make_identity(nc, ident)
            xT = const.tile([c.P, c.INT * c.NPC], F32R)
            nc.sync.dma_start(out=xT, in_=xT_d)
            win = const.tile([c.P, c.INT * c.H], F32R)
            nc.sync.dma_start(out=win, in_=win_d)
            wl = const.tile([c.P, c.L * c.HT * c.H], F32R)
            nc.sync.dma_start(out=wl, in_=wl_d)
            cin = const.tile([c.P, 3 * c.H], F32)     # b_in | g_in | beta_in bcast
            nc.sync.dma_start(out=cin, in_=cin_d)
            cl = const.tile([c.P, 3 * c.L * c.H], F32)  # bl | 0.9gl | 0.9betal bcast
            nc.sync.dma_start(out=cl, in_=cl_d)

            # ---- persistent state ----
            cur = state.tile([c.P, c.MT * c.H], F32)
            h0 = state.tile([c.P, c.MT * c.H], F32)
            curT = state.tile([c.P, c.HT * c.NPC], F32R)
            mpart = state.tile([c.P, c.MT * c.H], BF16)
            # m_full staging per AllGather part; part 0 is ping-ponged across
            # layers so its DMA carries a single wait (no WAR sem-chain) and
            # can start as soon as its AllGather lands, mid-previous-layer
            w0 = c.NCORES * (c.SPLITS[0][1] - c.SPLITS[0][0]) * c.H
            mfull0 = [state.tile([c.P, w0], BF16, name=f"mfull0_{par}",
                                 tag=f"mfull0_{par}") for par in (0, 1)]
            mfull_rest = {p: state.tile([c.P, c.NCORES * (e - s) * c.H], BF16,
                                        name=f"mfull{p}", tag=f"mfull{p}")
                          for p, (s, e) in enumerate(c.SPLITS) if p > 0}

            def mfull_of(l, p):
                return mfull0[l % 2] if p == 0 else mfull_rest[p]

            H = c.H

            def rsqrt_dve(out, ve):
                """out = (ve)^-0.5 on DVE only: exponent-hack seed + 3 Newton
                iterations. ve is [128, n] f32, strictly positive."""
                n = ve.shape[-1]
                i32 = statp.tile([c.P, n], mybir.dt.int32, tag="rs_i")
                # i = ve_bits >> 1
                nc.vector.tensor_scalar(out=i32, in0=ve.bitcast(mybir.dt.int32),
                                        scalar1=1, scalar2=None,
                                        op0=ALU.logical_shift_right)
                # i = 0x5f3759df - i  ==  i * (-1) + 0x5f3759df
                nc.vector.tensor_scalar(out=i32, in0=i32, scalar1=-1,
                                        scalar2=0x5F3759DF, op0=ALU.mult,
                                        op1=ALU.add)
                y = statp.tile([c.P, n], F32, tag="rs_y")
                nc.vector.tensor_copy(out=y, in_=i32.bitcast(F32))
                w = statp.tile([c.P, n], F32, tag="rs_w")
                for _ in range(2):
                    # w = ve * y * y ; y = y * (1.5 - 0.5 w)
                    nc.vector.tensor_tensor(out=w, in0=y, in1=y, op=ALU.mult)
                    nc.vector.tensor_tensor(out=w, in0=w, in1=ve, op=ALU.mult)
                    nc.vector.tensor_scalar(out=w, in0=w, scalar1=-0.5,
                                            scalar2=1.5, op0=ALU.mult, op1=ALU.add)
                    nc.vector.tensor_tensor(out=y, in0=y, in1=w, op=ALU.mult)
                nc.vector.tensor_copy(out=out, in_=y)

            def epilogue(m, ps, bias, gain, beta, first):
                """LN(gelu(ps + bias)) * gain + beta; first block writes h0/cur,
                layers do the residual update. Also refreshes curT for tile m."""
                t1 = tmp.tile([c.P, H], F32, tag="t1")
                nc.vector.tensor_tensor(out=t1, in0=ps, in1=bias, op=ALU.add)
                t2 = tmp.tile([c.P, H], F32, tag="t2")
                nc.scalar.activation(out=t2, in_=t1, func=c.ACT)
                st6 = statp.tile([c.P, 6], F32, tag="st6")
                nc.vector.bn_stats(out=st6, in_=t2)
                mv = statp.tile([c.P, 2], F32, tag="mv")
                nc.vector.bn_aggr(out=mv, in_=st6)
                ve = statp.tile([c.P, 1], F32, tag="ve")
                nc.vector.tensor_scalar_add(ve, mv[:, 1:2], c.EPS)
                rinv = statp.tile([c.P, 1], F32, tag="rinv")
                rsqrt_dve(rinv, ve)
                # z = (t2 - mean) * rinv
                z = tmp.tile([c.P, H], F32, tag="z")
                nc.vector.tensor_scalar(out=z, in0=t2, scalar1=mv[:, 0:1],
                                        scalar2=rinv, op0=ALU.subtract,
                                        op1=ALU.mult)
                nc.vector.tensor_tensor(out=z, in0=z, in1=gain, op=ALU.mult)
                cs = cur[:, m * H:(m + 1) * H]
                if first:
                    h0s = h0[:, m * H:(m + 1) * H]
                    nc.vector.tensor_tensor(out=h0s, in0=z, in1=beta, op=ALU.add)
                    nc.vector.tensor_copy(out=cs, in_=h0s)
                else:
                    nc.vector.tensor_tensor(out=z, in0=z, in1=beta, op=ALU.add)
                    # base = 0.1*h0 + cur ; cur = z + base
                    base = tmp.tile([c.P, H], F32, tag="base")
                    nc.vector.scalar_tensor_tensor(
                        out=base, in0=h0[:, m * H:(m + 1) * H], scalar=c.ALPHA,
                        in1=cs, op0=ALU.mult, op1=ALU.add)
                    nc.vector.tensor_tensor(out=cs, in0=z, in1=base, op=ALU.add)

            def refresh_curT(m):
                for t in range(c.HT):
                    pt = tpp.tile([c.P, c.P], F32, tag="tp")
                    nc.tensor.transpose(
                        pt, cur[:, m * H + t * 128: m * H + t * 128 + 128], ident)
                    nc.vector.tensor_copy(
                        out=curT[:, t * c.NPC + m * 128: t * c.NPC + (m + 1) * 128],
                        in_=pt)

            # helpers for the pipelined schedule -----------------------
            nparts = len(c.SPLITS)
            # phase p covers k-positions [poff[p], poff[p+1])
            poff = [0]
            for (s, e) in c.SPLITS:
                poff.append(poff[-1] + c.NCORES * (e - s))

            def produce_m(lnext, m):
                """GEMM1 for layer lnext on tile m (fp32) + bf16 cast."""
                ps = g1p.tile([c.P, H], F32, tag="g1")
                for t in range(c.HT):
                    nc.tensor.matmul(
                        ps,
                        lhsT=curT[:, t * c.NPC + m * 128:
                                  t * c.NPC + (m + 1) * 128],
                        rhs=wl[:, (lnext * c.HT + t) * H:
                               (lnext * c.HT + t + 1) * H],
                        start=(t == 0), stop=(t == c.HT - 1))
                nc.vector.tensor_copy(out=mpart[:, m * H:(m + 1) * H], in_=ps)

            def emit_ag(l, p):
                s, e = c.SPLITS[p]
                nc.sync.dma_start(out=cc_in[l][p].ap(),
                                  in_=mpart[:, s * H:e * H])
                nc.gpsimd.collective_compute(
                    "AllGather", ALU.bypass, replica_groups=rg,
                    ins=[cc_in[l][p].ap()], outs=[cc_out[l][p].ap()])

            def emit_mfull(l, p):
                s, e = c.SPLITS[p]
                w = (e - s) * c.H
                dst = mfull_of(l, p)
                for r in range(c.NCORES):
                    nc.sync.dma_start(
                        out=dst[:, r * w:(r + 1) * w],
                        in_=cc_out[l][p].ap()[r * c.P:(r + 1) * c.P, :])

            def tile_tail(l, m):
                """transpose + next-layer GEMM1 for tile m of layer l."""
                refresh_curT(m)
                produce_m(l + 1, m)

            def at_dma(ch, grp):
                t = atp.tile([c.P, c.KG * 256], BF16, tag="at")
                nc.sync.dma_start(out=t, in_=At_d[ch, grp])
                return t

            def phase_groups(p):
                return range(poff[p] // c.KG, poff[p + 1] // c.KG)

            # accumulators: both m-tiles of a chunk share one PSUM bank
            at_tiles = {}

            def mm_phase(l, pair, p, acc, atg):
                for ch in pair:
                    for g in phase_groups(p):
                        atb = atg.pop((ch, g))
                        for kk in range(c.KG):
                            j = g * c.KG + kk
                            rhs = mfull_of(l, c.KPART[j])[
                                :, c.KOFF[j] * H:(c.KOFF[j] + 1) * H]
                            nc.tensor.matmul(
                                acc[ch][0],
                                lhsT=atb[:, kk * 256: kk * 256 + 128],
                                rhs=rhs, start=(j == 0), stop=(j == c.KT - 1))
                            nc.tensor.matmul(
                                acc[ch][1],
                                lhsT=atb[:, kk * 256 + 128: kk * 256 + 256],
                                rhs=rhs, start=(j == 0), stop=(j == c.KT - 1))

            # ================= input block =================
            instage = []
            for m in range(c.MT):
                ps = g1p.tile([c.P, H], F32, tag="g1")
                for t in range(c.INT):
                    nc.tensor.matmul(
                        ps,
                        lhsT=xT[:, t * c.NPC + m * 128: t * c.NPC + (m + 1) * 128],
                        rhs=win[:, t * H:(t + 1) * H],
                        start=(t == 0), stop=(t == c.INT - 1))
                st = tmp.tile([c.P, H], F32, tag="instage", bufs=c.MT,
                              name=f"instage{m}")
                nc.vector.tensor_copy(out=st, in_=ps)
                instage.append(st)
            for p, (s, e) in enumerate(c.SPLITS):
                for m in range(s, e):
                    epilogue(m, instage[m], cin[:, 0:H], cin[:, H:2 * H],
                             cin[:, 2 * H:3 * H], first=True)
                    tile_tail(-1, m)   # produce_m uses lnext = 0
                emit_ag(0, p)
                emit_mfull(0, p)

            # ================= layers =================
            # Tail work (transpose+GEMM1+AG+mfull) is deferred so the PE never
            # waits on a DVE epilogue: each pair's tails are emitted between
            # the NEXT pair's matmul phases; the last pair's tails (pending)
            # land inside the next layer. mfull DMA placement avoids blocking
            # SP ahead of A^T prefetch pushes (see split_excess_waits).
            pending = None
            for l in range(c.L):
                last = l == c.L - 1
                lb = cl[:, (3 * l) * H:(3 * l + 1) * H]
                lg = cl[:, (3 * l + 1) * H:(3 * l + 2) * H]
                lbeta = cl[:, (3 * l + 2) * H:(3 * l + 3) * H]
                acc = {}

                def close_pair(pair):
                    for ch in pair:
                        for i in (0, 1):
                            m = 2 * ch + i
                            epilogue(m, acc[ch][i], lb, lg, lbeta,
                                     first=False)

                for pi, pair in enumerate(c.PAIRS):
                    for ch in pair:
                        acc[ch] = (aggp.tile([c.P, H], F32, tag="agg",
                                             name=f"acc_l{l}_c{ch}_0"),
                                   aggp.tile([c.P, H], F32, tag="agg",
                                             name=f"acc_l{l}_c{ch}_1"))
                    for p in range(nparts):
                        for ch in pair:
                            for g in phase_groups(p):
                                at_tiles[(ch, g)] = at_dma(ch, g)
                        if pi == 0 and pending is not None:
                            # deferred mfull parts for THIS layer's m (emitted
                            # after this phase's At pushes so SP blocking on
                            # the AG can't starve the A^T prefetch)
                            pl = pending[0]
                            if p == 0 and nparts >= 3:
                                emit_mfull(pl + 1, 1)
                            if p == min(1, nparts - 1):
                                for m in pending[1]:
                                    tile_tail(pl, m)
                                emit_ag(pl + 1, nparts - 1)
                                emit_mfull(pl + 1, nparts - 1)
                                pending = None
                        mm_phase(l, pair, p, acc, at_tiles)
                        if pi > 0 and p < len(c.PAIRS[pi - 1]) and not last:
                            chp = c.PAIRS[pi - 1][p]
                            for m in (2 * chp, 2 * chp + 1):
                                tile_tail(l, m)
                            if chp == c.PAIRS[pi - 1][-1] and len(c.PAIRS) > 1:
                                emit_ag(l + 1, pi - 1)
                    close_pair(pair)
                if not last:
                    lastpair = c.PAIRS[-1]
                    pending = (l, [m for ch in lastpair
                                   for m in (2 * ch, 2 * ch + 1)])
                    if nparts >= 2:
                        # part 0 feeds the next layer's first matmul phase;
                        # its WAR (vs this layer's phase-0 readers) has
                        # cleared by now, and its AG fired mid-layer
                        emit_mfull(l + 1, 0)

            # ================= output =================
            out_v = out_d.rearrange("(m p) q -> p m q", p=c.P)
            nc.sync.dma_start(out=out_v, in_=cur.rearrange("p (m q) -> p m q", q=H))

    if split_waits:
        split_excess_waits(nc, wsplit_sem)
    return nc


# ---------------------------------------------------------- host wrapper

def prep_inputs(cfg, x, edge_index, W_in, b_in, g_in, beta_in, Wl, bl, gl, betal):
    """Build the per-core input maps (numpy, host-side)."""
    c = cfg
    x = np.asarray(x, dtype=np.float32)
    edge_index = np.asarray(edge_index)
    W_in = np.asarray(W_in, dtype=np.float32)
    b_in = np.asarray(b_in, dtype=np.float32)
    g_in = np.asarray(g_in, dtype=np.float32)
    beta_in = np.asarray(beta_in, dtype=np.float32)
    Wl = np.asarray(Wl, dtype=np.float32)
    bl = np.asarray(bl, dtype=np.float32)
    gl = np.asarray(gl, dtype=np.float32)
    betal = np.asarray(betal, dtype=np.float32)

    N = c.N
    # GCN normalization with self-loops
    src = np.concatenate([edge_index[0], np.arange(N, dtype=np.int64)])
    dst = np.concatenate([edge_index[1], np.arange(N, dtype=np.int64)])
    deg = np.bincount(dst, minlength=N).astype(np.float32)
    dinv = np.where(deg > 0, deg ** -0.5, 0.0).astype(np.float32)
    w = dinv[dst] * dinv[src]

    # padded global ids: node n -> core r = n // RPC, slot i = n % RPC
    nn = np.arange(N, dtype=np.int64)
    gid = (nn // c.RPC) * c.NPC + (nn % c.RPC)
    A = np.zeros((c.NPAD, c.NPAD), dtype=np.float32)
    np.add.at(A, (gid[dst], gid[src]), w)
    A16 = A.astype(ml_dtypes.bfloat16)
    del A

    # per-layer constants, broadcast across partitions
    def bcast(v):  # [H] -> [128, H]
        return np.broadcast_to(np.asarray(v, np.float32), (c.P, v.shape[-1])).copy()

    cin = np.concatenate([bcast(b_in), bcast(g_in), bcast(beta_in)], axis=1)
    cl_list = []
    for l in range(c.L):
        cl_list += [bcast(bl[l]), bcast((1.0 - c.ALPHA) * gl[l]),
                    bcast((1.0 - c.ALPHA) * betal[l])]
    cl_h = np.concatenate(cl_list, axis=1)

    # Wl host layout: [128, L*HT*H], k-tile t of layer l at cols (l*HT+t)*H
    wl_h = np.zeros((c.P, c.L * c.HT * c.H), np.float32)
    for l in range(c.L):
        for t in range(c.HT):
            wl_h[:, (l * c.HT + t) * c.H:(l * c.HT + t + 1) * c.H] = \
                Wl[l][t * 128:(t + 1) * 128, :]

    win_h = np.zeros((c.P, c.INT * c.H), np.float32)
    for t in range(c.INT):
        win_h[:, t * c.H:(t + 1) * c.H] = W_in[t * 128:(t + 1) * 128, :]

    in_maps = []
    for r in range(c.NCORES):
        lo, hi = r * c.RPC, min((r + 1) * c.RPC, N)
        xp = np.zeros((c.NPC, c.IN), np.float32)
        xp[:hi - lo] = x[lo:hi]
        xT = np.zeros((c.P, c.INT * c.NPC), np.float32)
        for t in range(c.INT):
            xT[:, t * c.NPC:(t + 1) * c.NPC] = xp[:, t * 128:(t + 1) * 128].T

        blk = A16[r * c.NPC:(r + 1) * c.NPC, :]          # [NPC, NPAD] (dst, src)
        t = np.ascontiguousarray(blk.T)                   # [NPAD, NPC] (src, dst)
        # permute k-tiles into the device iteration order (part-major), then
        # -> [MC, G, P, KG*256]: chunk ch covers dst cols ch*256..,
        # group g covers k-positions g*KG..
        t = t.reshape(c.KT, 128, c.NPC)[c.PERM]
        at = t.reshape(c.G, c.KG, 128, c.MC, 256)
        at = at.transpose(3, 0, 2, 1, 4).reshape(c.MC, c.G, 128, c.KG * 256)
        at = np.ascontiguousarray(at)

        in_maps.append({
            "xT": xT, "At": at, "Win": win_h, "Wlh": wl_h,
            "cin": cin, "cl": cl_h,
        })
    return in_maps


def postprocess(cfg, results):
    c = cfg
    out = np.empty((c.N, c.H), np.float32)
    for r in range(c.NCORES):
        lo, hi = r * c.RPC, min((r + 1) * c.RPC, c.N)
        out[lo:hi] = results[r]["out"][:hi - lo]
    return out


_CACHE = {}
TRACE = False  # test harness sets True to capture an NTFF profile


def kernel(x, edge_index, W_in, b_in, g_in, beta_in, Wl, bl, gl, betal):
    from concourse import bass_utils
    cfg = Cfg()
    in_maps = prep_inputs(cfg, x, edge_index, W_in, b_in, g_in, beta_in,
                          Wl, bl, gl, betal)
    if "nc" not in _CACHE:
        _CACHE["nc"] = build_nc(cfg)
    res = bass_utils.run_bass_kernel_spmd(
        _CACHE["nc"], in_maps, core_ids=list(range(cfg.NCORES)), trace=TRACE)
    _CACHE["last_result"] = res
    return postprocess(cfg, res.results)
